# revision 55
# baseline (speedup 1.0000x reference)
"""Trainium2 Bass kernel for the DCNv4 bottleneck block.

Contract: kernel(**inputs) takes FULL unsharded inputs (as in reference
setup_inputs()) and returns the FULL (4, 256, 80, 80) fp32 output.

Sharding: 8 cores = 4 samples x 2 row-halves (40 rows each + halos).

Per-core pipeline (channel-major [C-part, flat-pixel] on an 84-wide frame,
all matmul operands bf16, fp32 PSUM accumulation):
  cv1 3x3 conv (9 shifted matmuls) + BN + SiLU               -> y1 [128, 3696]
  cv2 1x1 conv + BN + SiLU                                   -> y  [2][128, 3840]
  depthwise 3x3 (diag matmuls) + LayerNorm + GELU            -> dw  [2][128, 3456]
  combined in/out projection Xc = (out_w@in_w) y + bias      -> Xpm pixel-major [128, 30, 256]
  offset/mask projection (pixel-major) -> bilinear/mask coefficients
  deformable sampling as banded matmul: M^T built by GPSIMD local_scatter
  (bf16), PE-transposed to M chunks, out[t, c] = sum_q M[q, t] Xpm[q, c]
  BN3 + SiLU + residual, store channel-major.

The y buffer keeps a 2-column lead so Xpm chunk Q is exactly y columns
[128Q, 128Q+128): each out tile T samples q in [128T, 128T+468) -> 4 chunks.
"""

import numpy as np
import ml_dtypes
from contextlib import ExitStack

import concourse.bass as bass
import concourse.tile as tile
from concourse import bacc, mybir
from concourse import bass_utils
from concourse.ap import AP

f32 = mybir.dt.float32
bf16 = mybir.dt.bfloat16
i16 = mybir.dt.int16
AF = mybir.ActivationFunctionType
OP = mybir.AluOpType
AX = mybir.AxisListType

# ---- geometry constants ----
W = 84                  # frame width (80 image + 2 pad each side)
HX, HY, HD = 46, 44, 40
NPX = HX * W            # 3864  x frame pixels
NPY = HY * W            # 3696  y frame pixels
YL = 2                  # y buffer lead columns (y-pix p stored at col p+2)
YW = 3840               # y buffer width = XCH*128
NPD = HD * W            # 3360  out-region pixels
NTOT = 3456             # padded out pixels (27 tiles)
NT = 27                 # out-pixel tiles of 128
XCH = 30                # Xpm chunks of 128 (xpm q == y column q)
NCI = 4                 # M chunks per out tile (band q - t in [0, 468))
MTW = 512               # M^T row width
EPS_BN, EPS_LN = 1e-5, 1e-6

_BUILT = None  # cached (nc,)


def _build(dump=False):
    nc = bacc.Bacc("TRN2", target_bir_lowering=False, debug=False, num_devices=8)

    # ---------------- DRAM I/O ----------------
    d_x = nc.dram_tensor("xs", [4, 128, 2, NPX // 4], bf16, kind="ExternalInput")
    d_w1t = nc.dram_tensor("w1t", [128, 9, 2, 128], bf16, kind="ExternalInput")
    d_w2t = nc.dram_tensor("w2t", [128, 256], bf16, kind="ExternalInput")
    d_wct = nc.dram_tensor("wct", [128, 2, 256], bf16, kind="ExternalInput")
    d_bc = nc.dram_tensor("bcr", [1, 256], bf16, kind="ExternalInput")
    d_dwdg = nc.dram_tensor("dwdg", [128, 2, 9, 128], bf16, kind="ExternalInput")
    d_dwb = nc.dram_tensor("dwb", [2, 1, 128], bf16, kind="ExternalInput")
    d_womt = nc.dram_tensor("womt", [2, 128, 32], bf16, kind="ExternalInput")
    d_ombb = nc.dram_tensor("ombb", [128, 32], f32, kind="ExternalInput")
    d_s1 = nc.dram_tensor("s1", [128, 1], f32, kind="ExternalInput")
    d_t1 = nc.dram_tensor("t1", [128, 1], f32, kind="ExternalInput")
    d_s2 = nc.dram_tensor("s2", [2, 128, 1], f32, kind="ExternalInput")
    d_t2 = nc.dram_tensor("t2", [2, 128, 1], f32, kind="ExternalInput")
    d_lng = nc.dram_tensor("lng", [2, 128, 1], f32, kind="ExternalInput")
    d_lnb = nc.dram_tensor("lnb", [2, 128, 1], f32, kind="ExternalInput")
    d_ident = nc.dram_tensor("ident", [128, 128], f32, kind="ExternalInput")
    d_identb = nc.dram_tensor("identb", [128, 128], bf16, kind="ExternalInput")
    d_vmask = nc.dram_tensor("vmask", [128, XCH], f32, kind="ExternalInput")
    d_rowm = nc.dram_tensor("rowm", [2, 128, 1], f32, kind="ExternalInput")
    d_zeros = nc.dram_tensor("zeros", [128, 512], f32, kind="ExternalInput")
    d_xr = nc.dram_tensor("xr", [NT, 128, 256], bf16, kind="ExternalInput")
    d_t3b = nc.dram_tensor("t3b", [128, 256], f32, kind="ExternalInput")
    d_onesc = nc.dram_tensor("onesc", [128, 1], bf16, kind="ExternalInput")
    d_onesr = nc.dram_tensor("onesr", [1, 512], bf16, kind="ExternalInput")
    d_selm = nc.dram_tensor("selm", [32, NT * 128], bf16, kind="ExternalInput")
    d_out = nc.dram_tensor("out", [NT, 128, 256], bf16, kind="ExternalOutput")
    if dump:
        d_dy1 = nc.dram_tensor("dy1", [128, NPY], bf16, kind="ExternalOutput")
        d_dy = nc.dram_tensor("dy", [128, 2 * YW], bf16, kind="ExternalOutput")
        d_ddwg = nc.dram_tensor("ddwg", [128, 2 * NTOT], bf16, kind="ExternalOutput")
        d_dxpm = nc.dram_tensor("dxpm", [128, XCH * 256], bf16, kind="ExternalOutput")
        d_dcoef = nc.dram_tensor("dcoef", [128, NT * 32], f32, kind="ExternalOutput")
        d_dcbuf = nc.dram_tensor("dcbuf", [128, NT * 26], bf16, kind="ExternalOutput")

    with tile.TileContext(nc) as tc:
        with ExitStack() as ctx:
            P = ctx.enter_context(tc.tile_pool(name="persist", bufs=1))

            # ---------------- loads ----------------
            # big A-stage inputs on the gpsimd DMA queue (first to arrive);
            # everything else spread over the vector/scalar/sync queues
            x_sb = P.tile([128, 2, NPX], bf16)
            w1t = P.tile([128, 9, 2, 128], bf16)
            nc.gpsimd.dma_start(w1t[:], d_w1t.ap())
            SEG = NPX // 4
            for i in range(4):
                nc.gpsimd.dma_start(x_sb[:, :, SEG * i:SEG * (i + 1)], d_x.ap()[i])
            w2t = P.tile([128, 256], bf16)
            nc.scalar.dma_start(w2t[:], d_w2t.ap())
            wct = P.tile([128, 2, 256], bf16)
            nc.scalar.dma_start(wct[:], d_wct.ap())
            bcr = P.tile([1, 256], bf16)
            nc.scalar.dma_start(bcr[:], d_bc.ap())
            dwdg = P.tile([128, 2, 9, 128], bf16)
            nc.scalar.dma_start(dwdg[:], d_dwdg.ap())
            dwb = P.tile([1, 2, 128], bf16)
            nc.scalar.dma_start(dwb[:], d_dwb.ap().transpose([1, 0, 2]))
            womt = P.tile([128, 2, 32], bf16)
            nc.scalar.dma_start(womt[:], d_womt.ap().transpose([1, 0, 2]))
            ombb = P.tile([128, 32], f32)
            nc.sync.dma_start(ombb[:], d_ombb.ap())
            s1 = P.tile([128, 1], f32)
            nc.sync.dma_start(s1[:], d_s1.ap())
            t1 = P.tile([128, 1], f32)
            nc.sync.dma_start(t1[:], d_t1.ap())
            s2 = P.tile([128, 2], f32)
            nc.sync.dma_start(s2[:], d_s2.ap().transpose([1, 0, 2]))
            t2 = P.tile([128, 2], f32)
            nc.sync.dma_start(t2[:], d_t2.ap().transpose([1, 0, 2]))
            lng = P.tile([128, 2], f32)
            nc.sync.dma_start(lng[:], d_lng.ap().transpose([1, 0, 2]))
            lnb = P.tile([128, 2], f32)
            nc.sync.dma_start(lnb[:], d_lnb.ap().transpose([1, 0, 2]))
            ident = P.tile([128, 128], f32)
            nc.sync.dma_start(ident[:], d_ident.ap())
            identb = P.tile([128, 128], bf16)
            nc.sync.dma_start(identb[:], d_identb.ap())
            vmask = P.tile([128, XCH], f32)
            nc.sync.dma_start(vmask[:], d_vmask.ap())
            rowm = P.tile([128, 2], f32)
            nc.sync.dma_start(rowm[:], d_rowm.ap().transpose([1, 0, 2]))
            t3b = P.tile([128, 256], f32)
            nc.scalar.dma_start(t3b[:], d_t3b.ap())
            selm = P.tile([32, NT * 128], bf16)
            nc.scalar.dma_start(selm[:], d_selm.ap())

            zref = P.tile([128, 512], f32)
            nc.scalar.dma_start(zref[:], d_zeros.ap())

            def zero_cast(dst_ap):
                # DVE cast-copy zeros onto any view (verifier-clean)
                src = AP(zref.tensor, zref[:].offset,
                         [[512, dst_ap.ap[0][1]]] + [[0, d[1]] for d in dst_ap.ap[1:]])
                nc.vector.tensor_copy(dst_ap, src)

            ones_row = P.tile([1, 512], bf16)
            nc.gpsimd.dma_start(ones_row[:], d_onesr.ap())
            ones_col = P.tile([128, 1], bf16)
            nc.sync.dma_start(ones_col[:], d_onesc.ap())
            eps128 = P.tile([128, 1], f32)
            nc.vector.memset(eps128[:], EPS_LN)

            # scatter indices for M^T build: idx = t + 84u + v, u,v in [0,5)
            sidx = P.tile([128, 26], i16)
            nc.gpsimd.iota(sidx[:, 0:25], pattern=[[W, 5], [1, 5]], base=0,
                           channel_multiplier=1, allow_small_or_imprecise_dtypes=True)
            nc.vector.memset(sidx[:, 25:26], -1)

            # ---------------- persistent activations ----------------
            y1pool = tc.alloc_tile_pool(name="y1pool", bufs=1)
            y1 = y1pool.tile([128, NPY], bf16)
            y = P.tile([128, 2, YW], bf16)
            dw = P.tile([128, 2, NTOT], bf16)       # later overwritten by gelu output
            xpm = P.tile([128, XCH, 256], bf16)
            coefb = P.tile([128, NT, 32], f32)
            cbuf = P.tile([128, NT, 26], bf16)
            nc.vector.memset(cbuf[:], 0)

            # =============== stages A+B: cv1 + cv2, chunk-pipelined ===============
            # B(k) is emitted one chunk behind A(k+1) so the PE never waits on
            # A's SiLU (scalar).  The 1x1 cv2 needs exactly A's chunk range.
            with tc.tile_pool(name="psA", bufs=2, space="PSUM") as psA, \
                 tc.tile_pool(name="psB", bufs=2, space="PSUM") as psB:

                def emit_b(t0, nn):
                    for m in range(2):
                        ps = psB.tile([128, 512], f32, tag="pb")
                        nc.tensor.matmul(ps[:, :nn], w2t[:, 128 * m:128 * m + 128],
                                         y1[:, t0:t0 + nn], start=True, stop=True)
                        nc.scalar.activation(y[:, m, YL + t0:YL + t0 + nn], ps[:, :nn],
                                             AF.Silu, bias=t2[:, m:m + 1], scale=s2[:, m:m + 1])

                prev = None
                t0 = 1
                while t0 < NPY - 1:
                    nn = min(512, NPY - 1 - t0)
                    ps = psA.tile([128, 512], f32, tag="pa")
                    first = True
                    for ck in range(2):
                        for s in range(9):
                            ki, kj = s // 3, s % 3
                            off = ki * W + kj - 1
                            nc.tensor.matmul(
                                ps[:, :nn], w1t[:, s, ck, :],
                                x_sb[:, ck, t0 + off: t0 + off + nn],
                                start=first, stop=(ck == 1 and s == 8))
                            first = False
                    nc.scalar.activation(y1[:, t0:t0 + nn], ps[:, :nn], AF.Silu,
                                         bias=t1[:], scale=s1[:])
                    if prev is not None:
                        emit_b(*prev)
                    prev = (t0, nn)
                    t0 += nn
                emit_b(*prev)
            # (y pixels 0 and NPY-1 are pad columns -> zeroed just below)
            y1pool.release()
            # zero lead/tail and pad columns; zero out-of-image rows via rowmask
            for m in range(2):
                zero_cast(y[:, m, 0:YL])
                zero_cast(y[:, m, YL + NPY:YW])
                yv = AP(y.tensor, y[:].offset + m * YW + 2, [[2 * YW, 128], [W, HY], [1, 2]])
                zero_cast(yv)
                yv2 = AP(y.tensor, y[:].offset + m * YW + W, [[2 * YW, 128], [W, HY], [1, 2]])
                zero_cast(yv2)
                nc.vector.tensor_scalar(y[:, m, YL:YL + 2 * W], y[:, m, YL:YL + 2 * W],
                                        rowm[:, 0:1], None, OP.mult)
                nc.vector.tensor_scalar(y[:, m, YL + NPY - 2 * W:YL + NPY],
                                        y[:, m, YL + NPY - 2 * W:YL + NPY],
                                        rowm[:, 1:2], None, OP.mult)

            # =============== stage D: depthwise conv + LN + GELU ===============
            # (stage C is emitted later, interleaved with D3/E, so its matmuls
            # fill the PE while the DVE does the LN normalize work)
            # D1: depthwise conv (diag matmuls) over the full padded range; per-pixel
            # channel sums / sumsq via N=1 matmuls (pixel-major stats on 128 lanes).
            statb = P.tile([128, NT, 2], f32)
            with tc.tile_pool(name="psD", bufs=3, space="PSUM") as psD, \
                 tc.tile_pool(name="psS", bufs=2, space="PSUM") as psS, \
                 tc.tile_pool(name="dtmp", bufs=2) as dtmp:
                t0 = 0
                while t0 < NTOT:
                    nn = min(512, NTOT - t0)
                    sc = nn // 128
                    for m in range(2):
                        ps = psD.tile([128, 512], f32, tag="pdw")
                        for ss in range(9):
                            ki, kj = ss // 3, ss % 3
                            off = (ki + 1) * W + kj - 1 + YL
                            nc.tensor.matmul(ps[:, :nn], dwdg[:, m, ss, :],
                                             y[:, m, t0 + off: t0 + off + nn],
                                             start=(ss == 0), stop=False)
                        nc.tensor.matmul(ps[:, :nn], dwb[:, m, :], ones_row[:, :nn],
                                         start=False, stop=True)
                        nc.vector.tensor_copy(dw[:, m, t0:t0 + nn], ps[:, :nn])
                        sqm = dtmp.tile([128, 512], bf16, tag=f"sq{m}")
                        nc.scalar.activation(sqm[:, :nn], ps[:, :nn], AF.Square)
                        if m == 0:
                            sq0 = sqm
                        else:
                            sq1 = sqm
                    pst = psS.tile([128, 8], f32, tag="pstat")
                    for sub in range(sc):
                        sl = slice(t0 + 128 * sub, t0 + 128 * sub + 128)
                        nc.tensor.matmul(pst[:, 2 * sub:2 * sub + 1], dw[:, 0, sl],
                                         ones_col[:], start=True, stop=False)
                        nc.tensor.matmul(pst[:, 2 * sub:2 * sub + 1], dw[:, 1, sl],
                                         ones_col[:], start=False, stop=True)
                        nc.tensor.matmul(pst[:, 2 * sub + 1:2 * sub + 2],
                                         sq0[:, 128 * sub:128 * sub + 128],
                                         ones_col[:], start=True, stop=False)
                        nc.tensor.matmul(pst[:, 2 * sub + 1:2 * sub + 2],
                                         sq1[:, 128 * sub:128 * sub + 128],
                                         ones_col[:], start=False, stop=True)
                    nc.vector.tensor_copy(statb[:, t0 // 128: t0 // 128 + sc, :], pst[:, :2 * sc])
                    t0 += nn

            # D2: stats math on [128, NT] (all lanes), then transpose into a
            # packed [NT, 256] tile: row T = [rstd (128 px) | brow (128 px)]
            tsb = P.tile([32, 256], bf16)
            with tc.tile_pool(name="stt", bufs=1) as sttp, \
                 tc.tile_pool(name="psST", bufs=2, space="PSUM") as psST:
                st0 = AP(statb.tensor, statb[:].offset, [[NT * 2, 128], [2, NT]])
                st1 = AP(statb.tensor, statb[:].offset + 1, [[NT * 2, 128], [2, NT]])
                meanb = sttp.tile([128, NT], f32)
                nc.vector.tensor_scalar(meanb[:], st0, 1.0 / 256, None, OP.mult)
                ex2 = sttp.tile([128, NT], f32)
                nc.vector.tensor_scalar(ex2[:], st1, 1.0 / 256, None, OP.mult)
                msq = sttp.tile([128, NT], f32)
                nc.scalar.activation(msq[:], meanb[:], AF.Square)
                nc.vector.tensor_tensor(ex2[:], ex2[:], msq[:], OP.subtract)
                sdev = sttp.tile([128, NT], f32)
                nc.scalar.activation(sdev[:], ex2[:], AF.Sqrt, bias=eps128[:], scale=1.0)
                rstdb = sttp.tile([128, NT], f32)
                with nc.allow_low_precision(reason="LN rstd"):
                    nc.vector.reciprocal(rstdb[:], sdev[:])
                browb = sttp.tile([128, NT], f32)
                nc.vector.scalar_tensor_tensor(browb[:], meanb[:], -1.0, rstdb[:],
                                               OP.mult, OP.mult)
                for ci, src in ((0, rstdb), (1, browb)):
                    pT = psST.tile([128, 128], f32, tag="pT")
                    nc.tensor.transpose(pT[:NT, :], src[:], ident[:])
                    nc.vector.tensor_copy(tsb[:NT, 128 * ci:128 * ci + 128], pT[:NT, :])

            # =============== stages D3 + C + E + coefficients + F, one pipeline ===============
            # D3 (LN normalize + gelu) interleaves with stage C chunks (PE
            # filler); then one full-size softmax/bilinear-prep pass (single
            # Exp table load); then per tile group the bilinear accumulation
            # (gpsimd mults + DVE strided adds) interleaves with stage F.
            GROUPS = [(0, 2), (2, 7), (7, 13), (13, 20), (20, 27)]

            pipe_ctx = ExitStack()
            with pipe_ctx:
                cfp = pipe_ctx.enter_context(tc.tile_pool(name="cf", bufs=1))
                mtp = pipe_ctx.enter_context(tc.tile_pool(name="mtp", bufs=2))
                msbp = pipe_ctx.enter_context(tc.tile_pool(name="msb", bufs=2))
                fin = pipe_ctx.enter_context(tc.tile_pool(name="fin", bufs=3))
                dtmp2 = pipe_ctx.enter_context(tc.tile_pool(name="dtmp2", bufs=3))
                de_ctx = ExitStack()
                psAB = de_ctx.enter_context(tc.tile_pool(name="psAB", bufs=2, space="PSUM"))
                psE = de_ctx.enter_context(tc.tile_pool(name="psE", bufs=2, space="PSUM"))
                psC = de_ctx.enter_context(tc.tile_pool(name="psC", bufs=2, space="PSUM"))

                def emit_c_chunk(Q):
                    p0 = 128 * Q
                    ps = psC.tile([128, 256], f32, tag="pc")
                    nc.tensor.matmul(ps[:], y[:, 0, p0:p0 + 128], wct[:, 0, :],
                                     start=True, stop=False)
                    nc.tensor.matmul(ps[:], y[:, 1, p0:p0 + 128], wct[:, 1, :],
                                     start=False, stop=False)
                    nc.tensor.matmul(ps[:], ones_row[:, 0:128], bcr[:],
                                     start=False, stop=True)
                    nc.vector.tensor_scalar(xpm[:, Q, :], ps[:],
                                            vmask[:, Q:Q + 1], None, OP.mult)

                def emit_d3_pair(T0, npair):
                    # rstd/brow broadcast via sel-matrix matmul
                    # (ps[i, j] = sum_k sel[k, T*128+i] tsb[k, j] = tsb[T, j])
                    t0 = 128 * T0
                    nn = 128 * npair
                    ps = psAB.tile([128, 512], f32, tag="pab")
                    for i in range(npair):
                        nc.tensor.matmul(ps[:, 256 * i:256 * i + 256],
                                         selm[:NT, t0 + 128 * i:t0 + 128 * (i + 1)],
                                         tsb[:NT, :], start=True, stop=True)
                    pa = AP(ps.tensor, ps[:].offset, [[512, 128], [0, 2], [256, npair], [1, 128]])
                    pb = AP(ps.tensor, ps[:].offset + 128, [[512, 128], [0, 2], [256, npair], [1, 128]])
                    zt = dtmp2.tile([128, 512], f32, tag="zt")
                    zv = AP(zt.tensor, zt[:].offset, [[512, 128], [256, 2], [128, npair], [1, 128]])
                    dwv = AP(dw.tensor, dw[:].offset + t0,
                             [[2 * NTOT, 128], [NTOT, 2], [128, npair], [1, 128]])
                    nc.vector.tensor_tensor(zv, dwv, pa, OP.mult)
                    nc.vector.tensor_tensor(zv, zv, pb, OP.add)
                    for m in range(2):
                        nc.scalar.activation(dw[:, m, t0:t0 + nn], zt[:, 256 * m:256 * m + nn],
                                             AF.Gelu, bias=lnb[:, m:m + 1], scale=lng[:, m:m + 1])

                def emit_e_tile(T):
                    ps = psE.tile([128, 32], f32, tag="pe")
                    nc.tensor.matmul(ps[:], dw[:, 0, 128 * T:128 * T + 128], womt[:, 0, :],
                                     start=True, stop=False)
                    nc.tensor.matmul(ps[:], dw[:, 1, 128 * T:128 * T + 128], womt[:, 1, :],
                                     start=False, stop=True)
                    nc.vector.tensor_tensor(coefb[:, T, :], ps[:], ombb[:], OP.add)

                cf_t = {}

                def emit_cf_phase1():
                    # softmax over masks + bilinear weight prep, full NT width
                    # (one Exp table load, big DVE ops)
                    cf = coefb[:].offset
                    cten = coefb.tensor

                    def cview(col0, step, cnt=9):
                        return AP(cten, cf + col0, [[NT * 32, 128], [32, NT], [step, cnt]])

                    ox = cview(0, 2)
                    oy = cview(1, 2)
                    lg = cview(18, 1)
                    t = cf_t
                    for nm in ("msm", "ix", "iy", "lx", "ly", "wx0", "wy0", "mx0",
                               "my0", "ta", "tb", "p00", "p01", "p10", "p11",
                               "gt", "contrib0", "contrib1"):
                        t[nm] = cfp.tile([128, NT, 9], bf16, tag=nm, name=nm)
                    mx = cfp.tile([128, NT], bf16, tag="mx")
                    nc.vector.tensor_reduce(mx[:], lg, axis=AX.X, op=OP.max)
                    E = cfp.tile([128, NT, 9], bf16, tag="E")
                    mxb = AP(mx.tensor, mx[:].offset, [[NT, 128], [1, NT], [0, 9]])
                    nc.vector.tensor_tensor(E[:], lg, mxb, OP.subtract)
                    nc.scalar.activation(E[:], E[:], AF.Exp)
                    se = cfp.tile([128, NT], bf16, tag="se")
                    with nc.allow_low_precision(reason="bf16 softmax"):
                        nc.vector.tensor_reduce(se[:], E[:], axis=AX.X, op=OP.add)
                        rs = cfp.tile([128, NT], bf16, tag="rs")
                        nc.vector.reciprocal(rs[:], se[:])
                    rsb = AP(rs.tensor, rs[:].offset, [[NT, 128], [1, NT], [0, 9]])
                    nc.vector.tensor_tensor(t["msm"][:], E[:], rsb, OP.mult)
                    # fractional parts and floor indicators
                    nc.vector.tensor_scalar(t["ix"][:], ox, 0.0, None, OP.is_lt)
                    nc.vector.tensor_scalar(t["iy"][:], oy, 0.0, None, OP.is_lt)
                    nc.vector.tensor_tensor(t["lx"][:], ox, t["ix"][:], OP.add)
                    nc.vector.tensor_tensor(t["ly"][:], oy, t["iy"][:], OP.add)
                    nc.vector.tensor_scalar(t["wx0"][:], t["lx"][:], -1.0, 1.0, OP.mult, OP.add)
                    nc.vector.tensor_scalar(t["wy0"][:], t["ly"][:], -1.0, 1.0, OP.mult, OP.add)
                    nc.vector.tensor_scalar(t["mx0"][:], t["ix"][:], -1.0, 1.0, OP.mult, OP.add)
                    nc.vector.tensor_scalar(t["my0"][:], t["iy"][:], -1.0, 1.0, OP.mult, OP.add)
                    nc.vector.tensor_tensor(t["ta"][:], t["msm"][:], t["wy0"][:], OP.mult)
                    nc.vector.tensor_tensor(t["tb"][:], t["msm"][:], t["ly"][:], OP.mult)
                    for a, tv in ((0, "ta"), (1, "tb")):
                        for b, wv in ((0, "wx0"), (1, "lx")):
                            nc.vector.tensor_tensor(t[f"p{a}{b}"][:], t[tv][:],
                                                    t[wv][:], OP.mult)

                def emit_cf_group(g0, GRP):
                    # bilinear 5x5 accumulation for tiles [g0, g0+GRP):
                    # contiguous mults on gpsimd, strided adds on DVE
                    t = cf_t

                    def sl(nm):
                        return t[nm][:, g0:g0 + GRP, :]

                    k = 0
                    for sy, myv in ((0, "my0"), (1, "iy")):
                        for sx, mxv in ((0, "mx0"), (1, "ix")):
                            nc.vector.tensor_tensor(sl("gt"), sl(myv), sl(mxv), OP.mult)
                            for a, b in ((0, 0), (0, 1), (1, 0), (1, 1)):
                                u0 = 1 + a - sy
                                v0 = 1 + b - sx
                                cb = t[f"contrib{k % 2}"]
                                k += 1
                                nc.vector.tensor_tensor(cb[:, g0:g0 + GRP, :],
                                                        sl(f"p{a}{b}"), sl("gt"), OP.mult)
                                # C5[:, :, u0 + j, v0 + i] += contrib[i, j]
                                dstv = AP(cbuf.tensor,
                                          cbuf[:].offset + 26 * g0 + (u0 * 5 + v0),
                                          [[NT * 26, 128], [26, GRP], [1, 3], [5, 3]])
                                srcv = AP(cb.tensor, cb[:].offset + 9 * g0,
                                          [[NT * 9, 128], [9, GRP], [3, 3], [1, 3]])
                                nc.vector.tensor_tensor(dstv, dstv, srcv, OP.add)

                def emit_f_tile(T, psT, psZ):
                    mt = mtp.tile([128, MTW], bf16, tag="mt")
                    nc.gpsimd.local_scatter(mt[:], cbuf[:, T, :],
                                            sidx[:], channels=128, num_elems=MTW,
                                            num_idxs=26)
                    msb = msbp.tile([128, NCI, 128], bf16, tag="msb")
                    for cp in range(NCI // 2):
                        pst = psT.tile([128, 256], bf16, tag="pst")
                        for j in range(2):
                            ci = 2 * cp + j
                            nc.tensor.transpose(pst[:, 128 * j:128 * j + 128],
                                                mt[:, 128 * ci:128 * ci + 128], identb[:])
                        if cp == 0:
                            nc.vector.tensor_copy(msb[:, 2 * cp:2 * cp + 2, :], pst[:])
                        else:
                            nc.scalar.copy(msb[:, 2 * cp:2 * cp + 2, :], pst[:])
                    psz = psZ.tile([128, 256], f32, tag="psz")
                    for ci in range(NCI):
                        nc.tensor.matmul(psz[:], msb[:, ci, :], xpm[:, T + ci, :],
                                         start=(ci == 0), stop=(ci == NCI - 1))
                    xrt = fin.tile([128, 256], bf16, tag="xrt")
                    nc.sync.dma_start(xrt[:], d_xr.ap()[T])
                    zb = fin.tile([128, 256], f32, tag="zb")
                    nc.vector.tensor_tensor(zb[:], psz[:], t3b[:], OP.add)
                    zact = fin.tile([128, 256], f32, tag="zact")
                    nc.scalar.activation(zact[:], zb[:], AF.Silu)
                    osb = fin.tile([128, 256], bf16, tag="osb")
                    nc.vector.tensor_tensor(osb[:], zact[:], xrt[:], OP.add)
                    nc.sync.dma_start(d_out.ap()[T], osb[:])

                # D3 pairs interleaved with C chunks and E tiles (C's matmuls
                # keep the PE busy while the DVE normalizes)
                ptr_c = 0
                for T0 in range(0, NT, 2):
                    npair = min(2, NT - T0)
                    emit_d3_pair(T0, npair)
                    for _ in range(2):
                        if ptr_c < 24:
                            emit_c_chunk(ptr_c)
                            ptr_c += 1
                    for T in range(T0, T0 + npair):
                        emit_e_tile(T)

                emit_cf_phase1()
                # remaining Xpm chunks: PE filler under phase1's DVE work
                while ptr_c < XCH:
                    emit_c_chunk(ptr_c)
                    ptr_c += 1
                de_ctx.close()
                with tc.tile_pool(name="psT", bufs=2, space="PSUM") as psT, \
                     tc.tile_pool(name="psZ", bufs=3, space="PSUM") as psZ:
                    for ga, gb in GROUPS:
                        emit_cf_group(ga, gb - ga)
                        for T in range(ga, gb):
                            emit_f_tile(T, psT, psZ)

            if dump:
                nc.sync.dma_start(d_dy1.ap(), y1[:])
                nc.sync.dma_start(d_dy.ap(), y[:].rearrange("p a b -> p (a b)"))
                nc.sync.dma_start(d_ddwg.ap(), dw[:].rearrange("p a b -> p (a b)"))
                nc.sync.dma_start(d_dxpm.ap(), xpm[:].rearrange("p a b -> p (a b)"))
                nc.sync.dma_start(d_dcoef.ap(), coefb[:].rearrange("p a b -> p (a b)"))
                nc.sync.dma_start(d_dcbuf.ap(), cbuf[:].rearrange("p a b -> p (a b)"))

    nc.compile()
    return nc


def _get_built():
    global _BUILT
    if _BUILT is None:
        _BUILT = _build()
    return _BUILT


def _bf(a):
    return np.asarray(a, dtype=ml_dtypes.bfloat16)


def _prep(inputs):
    g = {k: np.asarray(v, dtype=np.float32) for k, v in inputs.items()}
    x = g["x"]

    s1 = g["g1"] / np.sqrt(g["v1"] + EPS_BN)
    t1 = g["b1"] - g["m1"] * s1
    s2 = g["g2"] / np.sqrt(g["v2"] + EPS_BN)
    t2 = g["b2"] - g["m2"] * s2
    s3 = g["g3"] / np.sqrt(g["v3"] + EPS_BN)
    t3 = g["b3"] - g["m3"] * s3

    w1 = g["w1"]  # [128, 256, 3, 3]
    w1t = np.zeros((9, 2, 128, 128), np.float32)
    for ki in range(3):
        for kj in range(3):
            for ck in range(2):
                w1t[ki * 3 + kj, ck] = w1[:, 128 * ck:128 * ck + 128, ki, kj].T
    w2t = g["w2"][:, :, 0, 0].T.copy()  # [128, 256]
    Wc = g["out_w"] @ g["in_w"]
    wct = np.stack([Wc.T[:128], Wc.T[128:]])  # [2, 128, 256]
    bc = (g["out_w"] @ g["in_b"] + g["out_b"])[None, :]  # [1, 256]
    dwdg = np.zeros((2, 9, 128, 128), np.float32)
    for ck in range(2):
        for s in range(9):
            np.fill_diagonal(dwdg[ck, s], g["dw_w"][128 * ck:128 * ck + 128, 0, s // 3, s % 3])
    dwb = np.zeros((2, 1, 128), np.float32)
    dwb[0, 0] = g["dw_b"][:128]
    dwb[1, 0] = g["dw_b"][128:]
    womt = np.zeros((2, 128, 32), np.float32)
    for ck in range(2):
        womt[ck, :, :18] = g["off_w"][:, 128 * ck:128 * ck + 128].T
        womt[ck, :, 18:27] = g["msk_w"][:, 128 * ck:128 * ck + 128].T
    ombb = np.zeros((128, 32), np.float32)
    ombb[:, :18] = g["off_b"][None, :]
    ombb[:, 18:27] = g["msk_b"][None, :]
    ident = np.eye(128, dtype=np.float32)

    def colsplit(v):  # [256] -> [2, 128, 1]
        return v.reshape(2, 128, 1).astype(np.float32)

    # fold BN3 scale into the combined projection; t3 added on-chip
    wct = (wct.reshape(2, 128, 256) * s3[None, None, :]).astype(np.float32)
    bc = (bc * s3[None, :]).astype(np.float32)

    selm = np.zeros((32, NT * 128), np.float32)
    for T in range(NT):
        selm[T, 128 * T:128 * T + 128] = 1.0

    shared = dict(
        zeros=np.zeros((128, 512), np.float32),
        onesc=_bf(np.ones((128, 1))),
        onesr=_bf(np.ones((1, 512))),
        selm=_bf(selm),
        w1t=_bf(w1t.transpose(2, 0, 1, 3).copy()), w2t=_bf(w2t),
        wct=_bf(wct.transpose(1, 0, 2).copy()), bcr=_bf(bc),
        dwdg=_bf(dwdg.transpose(2, 0, 1, 3).copy()), dwb=_bf(dwb),
        womt=_bf(womt), ombb=ombb,
        s1=s1[:, None], t1=t1[:, None],
        s2=colsplit(s2), t2=colsplit(t2),
        lng=colsplit(g["ln_g"]), lnb=colsplit(g["ln_b"]),
        ident=ident, identb=_bf(ident),
        t3b=np.broadcast_to(t3[None, :], (128, 256)).copy(),
    )

    in_maps = []
    for c in range(8):
        n, h = c // 2, c % 2
        r0 = 40 * h - 3  # x frame row 0 in global coords
        xs = np.zeros((2, 128, HX, W), np.float32)
        glo = max(r0, 0)
        ghi = min(r0 + HX, 80)
        xs[0, :, glo - r0:ghi - r0, 2:82] = x[n, :128, glo:ghi, :]
        xs[1, :, glo - r0:ghi - r0, 2:82] = x[n, 128:, glo:ghi, :]
        # validity mask for Xpm pixels: q = 128*Q + p -> y-pix q - YL
        vm = np.zeros((XCH * 128,), np.float32)
        qs = np.arange(XCH * 128)
        pix = qs - YL
        rv, cv = pix // W, pix % W
        gr = 40 * h + rv - 2
        ok = (pix >= 0) & (pix < NPY) & (cv >= 2) & (cv < 82) & (gr >= 0) & (gr < 80)
        vm[ok] = 1.0
        vmask = vm.reshape(XCH, 128).T.copy()  # [128, XCH]
        rowm = np.zeros((2, 128, 1), np.float32)
        rowm[0] = 0.0 if h == 0 else 1.0   # y rows [0,2) valid only for h=1
        rowm[1] = 1.0 if h == 0 else 0.0   # y rows [42,44) valid only for h=0
        # pixel-major residual input: xr[T, p, c] = x at out-frame pixel 128T+p
        xflat = np.concatenate([xs[0], xs[1]], 0).reshape(256, HX, W)
        xres = xflat[:, 3:43, :].reshape(256, NPD).T  # [NPD, 256]
        xr = np.zeros((NT * 128, 256), np.float32)
        xr[:NPD] = xres
        m = dict(shared)
        m["xr"] = _bf(xr.reshape(NT, 128, 256))
        m["xs"] = _bf(xs.reshape(2, 128, 4, NPX // 4).transpose(2, 1, 0, 3).copy())
        m["vmask"] = vmask
        m["rowm"] = rowm
        in_maps.append(m)
    return in_maps


def kernel(**inputs):
    nc = _get_built()
    in_maps = _prep(inputs)
    res = bass_utils.run_bass_kernel_spmd(nc, in_maps, core_ids=list(range(8)))
    out = np.zeros((4, 256, 80, 80), np.float32)
    for c in range(8):
        n, h = c // 2, c % 2
        o = np.asarray(res.results[c]["out"], np.float32).reshape(NT * 128, 256)[:NPD]
        o = o.reshape(HD, W, 256)[:, 2:82].transpose(2, 0, 1)
        out[n, :, 40 * h:40 * h + 40, :] = o
    return out


# revision 56
# speedup vs baseline: 1.0236x; 1.0236x over previous
"""Trainium2 Bass kernel for the DCNv4 bottleneck block.

Contract: kernel(**inputs) takes FULL unsharded inputs (as in reference
setup_inputs()) and returns the FULL (4, 256, 80, 80) fp32 output.

Sharding: 8 cores = 4 samples x 2 row-halves (40 rows each + halos).

Per-core pipeline (channel-major [C-part, flat-pixel] on an 84-wide frame,
all matmul operands bf16, fp32 PSUM accumulation):
  cv1 3x3 conv (9 shifted matmuls) + BN + SiLU               -> y1 [128, 3696]
  cv2 1x1 conv + BN + SiLU                                   -> y  [2][128, 3840]
  depthwise 3x3 (diag matmuls) + LayerNorm + GELU            -> dw  [2][128, 3456]
  combined in/out projection Xc = (out_w@in_w) y + bias      -> Xpm pixel-major [128, 30, 256]
  offset/mask projection (pixel-major) -> bilinear/mask coefficients
  deformable sampling as banded matmul: M^T built by GPSIMD local_scatter
  (bf16), PE-transposed to M chunks, out[t, c] = sum_q M[q, t] Xpm[q, c]
  BN3 + SiLU + residual, store channel-major.

The y buffer keeps a 2-column lead so Xpm chunk Q is exactly y columns
[128Q, 128Q+128): each out tile T samples q in [128T, 128T+468) -> 4 chunks.
"""

import numpy as np
import ml_dtypes
from contextlib import ExitStack

import concourse.bass as bass
import concourse.tile as tile
from concourse import bacc, mybir
from concourse import bass_utils
from concourse.ap import AP

f32 = mybir.dt.float32
bf16 = mybir.dt.bfloat16
i16 = mybir.dt.int16
AF = mybir.ActivationFunctionType
OP = mybir.AluOpType
AX = mybir.AxisListType

# ---- geometry constants ----
W = 84                  # frame width (80 image + 2 pad each side)
HX, HY, HD = 46, 44, 40
NPX = HX * W            # 3864  x frame pixels
NPY = HY * W            # 3696  y frame pixels
YL = 2                  # y buffer lead columns (y-pix p stored at col p+2)
YW = 3840               # y buffer width = XCH*128
NPD = HD * W            # 3360  out-region pixels
NTOT = 3456             # padded out pixels (27 tiles)
NT = 27                 # out-pixel tiles of 128
XCH = 30                # Xpm chunks of 128 (xpm q == y column q)
NCI = 4                 # M chunks per out tile (band q - t in [0, 468))
MTW = 512               # M^T row width
EPS_BN, EPS_LN = 1e-5, 1e-6

_BUILT = None  # cached (nc,)


def _build(dump=False):
    nc = bacc.Bacc("TRN2", target_bir_lowering=False, debug=False, num_devices=8)

    # ---------------- DRAM I/O ----------------
    d_x = nc.dram_tensor("xs", [4, 128, 2, NPX // 4], bf16, kind="ExternalInput")
    d_w1t = nc.dram_tensor("w1t", [128, 9, 2, 128], bf16, kind="ExternalInput")
    d_w2t = nc.dram_tensor("w2t", [128, 256], bf16, kind="ExternalInput")
    d_wct = nc.dram_tensor("wct", [128, 2, 256], bf16, kind="ExternalInput")
    d_bc = nc.dram_tensor("bcr", [1, 256], bf16, kind="ExternalInput")
    d_dwdg = nc.dram_tensor("dwdg", [128, 2, 9, 128], bf16, kind="ExternalInput")
    d_dwb = nc.dram_tensor("dwb", [2, 1, 128], bf16, kind="ExternalInput")
    d_womt = nc.dram_tensor("womt", [2, 128, 32], bf16, kind="ExternalInput")
    d_ombb = nc.dram_tensor("ombb", [128, 32], f32, kind="ExternalInput")
    d_s1 = nc.dram_tensor("s1", [128, 1], f32, kind="ExternalInput")
    d_t1 = nc.dram_tensor("t1", [128, 1], f32, kind="ExternalInput")
    d_s2 = nc.dram_tensor("s2", [2, 128, 1], f32, kind="ExternalInput")
    d_t2 = nc.dram_tensor("t2", [2, 128, 1], f32, kind="ExternalInput")
    d_lng = nc.dram_tensor("lng", [2, 128, 1], f32, kind="ExternalInput")
    d_lnb = nc.dram_tensor("lnb", [2, 128, 1], f32, kind="ExternalInput")
    d_ident = nc.dram_tensor("ident", [128, 128], f32, kind="ExternalInput")
    d_identb = nc.dram_tensor("identb", [128, 128], bf16, kind="ExternalInput")
    d_vmask = nc.dram_tensor("vmask", [128, XCH], f32, kind="ExternalInput")
    d_rowm = nc.dram_tensor("rowm", [2, 128, 1], f32, kind="ExternalInput")
    d_zeros = nc.dram_tensor("zeros", [128, 512], f32, kind="ExternalInput")
    d_xr = nc.dram_tensor("xr", [NT, 128, 256], bf16, kind="ExternalInput")
    d_t3r = nc.dram_tensor("t3r", [1, 256], bf16, kind="ExternalInput")
    d_onesc = nc.dram_tensor("onesc", [128, 1], bf16, kind="ExternalInput")
    d_onesr = nc.dram_tensor("onesr", [1, 512], bf16, kind="ExternalInput")
    d_selm = nc.dram_tensor("selm", [32, NT * 128], bf16, kind="ExternalInput")
    d_out = nc.dram_tensor("out", [NT, 128, 256], bf16, kind="ExternalOutput")
    if dump:
        d_dy1 = nc.dram_tensor("dy1", [128, NPY], bf16, kind="ExternalOutput")
        d_dy = nc.dram_tensor("dy", [128, 2 * YW], bf16, kind="ExternalOutput")
        d_ddwg = nc.dram_tensor("ddwg", [128, 2 * NTOT], bf16, kind="ExternalOutput")
        d_dxpm = nc.dram_tensor("dxpm", [128, XCH * 256], bf16, kind="ExternalOutput")
        d_dcoef = nc.dram_tensor("dcoef", [128, NT * 32], f32, kind="ExternalOutput")
        d_dcbuf = nc.dram_tensor("dcbuf", [128, NT * 26], bf16, kind="ExternalOutput")

    with tile.TileContext(nc) as tc:
        with ExitStack() as ctx:
            P = ctx.enter_context(tc.tile_pool(name="persist", bufs=1))

            # ---------------- loads ----------------
            # big A-stage inputs on the gpsimd DMA queue (first to arrive);
            # everything else spread over the vector/scalar/sync queues
            x_sb = P.tile([128, 2, NPX], bf16)
            w1t = P.tile([128, 9, 2, 128], bf16)
            nc.gpsimd.dma_start(w1t[:], d_w1t.ap())
            SEG = NPX // 4
            for i in range(4):
                nc.gpsimd.dma_start(x_sb[:, :, SEG * i:SEG * (i + 1)], d_x.ap()[i])
            w2t = P.tile([128, 256], bf16)
            nc.scalar.dma_start(w2t[:], d_w2t.ap())
            wct = P.tile([128, 2, 256], bf16)
            nc.scalar.dma_start(wct[:], d_wct.ap())
            bcr = P.tile([1, 256], bf16)
            nc.scalar.dma_start(bcr[:], d_bc.ap())
            dwdg = P.tile([128, 2, 9, 128], bf16)
            nc.scalar.dma_start(dwdg[:], d_dwdg.ap())
            dwb = P.tile([1, 2, 128], bf16)
            nc.scalar.dma_start(dwb[:], d_dwb.ap().transpose([1, 0, 2]))
            womt = P.tile([128, 2, 32], bf16)
            nc.scalar.dma_start(womt[:], d_womt.ap().transpose([1, 0, 2]))
            ombb = P.tile([128, 32], f32)
            nc.sync.dma_start(ombb[:], d_ombb.ap())
            s1 = P.tile([128, 1], f32)
            nc.sync.dma_start(s1[:], d_s1.ap())
            t1 = P.tile([128, 1], f32)
            nc.sync.dma_start(t1[:], d_t1.ap())
            s2 = P.tile([128, 2], f32)
            nc.sync.dma_start(s2[:], d_s2.ap().transpose([1, 0, 2]))
            t2 = P.tile([128, 2], f32)
            nc.sync.dma_start(t2[:], d_t2.ap().transpose([1, 0, 2]))
            lng = P.tile([128, 2], f32)
            nc.sync.dma_start(lng[:], d_lng.ap().transpose([1, 0, 2]))
            lnb = P.tile([128, 2], f32)
            nc.sync.dma_start(lnb[:], d_lnb.ap().transpose([1, 0, 2]))
            ident = P.tile([128, 128], f32)
            nc.sync.dma_start(ident[:], d_ident.ap())
            identb = P.tile([128, 128], bf16)
            nc.sync.dma_start(identb[:], d_identb.ap())
            vmask = P.tile([128, XCH], f32)
            nc.sync.dma_start(vmask[:], d_vmask.ap())
            rowm = P.tile([128, 2], f32)
            nc.sync.dma_start(rowm[:], d_rowm.ap().transpose([1, 0, 2]))
            t3r = P.tile([1, 256], bf16)
            nc.scalar.dma_start(t3r[:], d_t3r.ap())
            selm = P.tile([32, NT * 128], bf16)
            nc.scalar.dma_start(selm[:], d_selm.ap())

            zref = P.tile([128, 512], f32)
            nc.scalar.dma_start(zref[:], d_zeros.ap())

            def zero_cast(dst_ap):
                # DVE cast-copy zeros onto any view (verifier-clean)
                src = AP(zref.tensor, zref[:].offset,
                         [[512, dst_ap.ap[0][1]]] + [[0, d[1]] for d in dst_ap.ap[1:]])
                nc.vector.tensor_copy(dst_ap, src)

            ones_row = P.tile([1, 512], bf16)
            nc.gpsimd.dma_start(ones_row[:], d_onesr.ap())
            ones_col = P.tile([128, 1], bf16)
            nc.sync.dma_start(ones_col[:], d_onesc.ap())
            eps128 = P.tile([128, 1], f32)
            nc.vector.memset(eps128[:], EPS_LN)

            # scatter indices for M^T build: idx = t + 84u + v, u,v in [0,5)
            sidx = P.tile([128, 26], i16)
            nc.gpsimd.iota(sidx[:, 0:25], pattern=[[W, 5], [1, 5]], base=0,
                           channel_multiplier=1, allow_small_or_imprecise_dtypes=True)
            nc.vector.memset(sidx[:, 25:26], -1)

            # ---------------- persistent activations ----------------
            y1pool = tc.alloc_tile_pool(name="y1pool", bufs=1)
            y1 = y1pool.tile([128, NPY], bf16)
            y = P.tile([128, 2, YW], bf16)
            dw = P.tile([128, 2, NTOT], bf16)       # later overwritten by gelu output
            xpm = P.tile([128, XCH, 256], bf16)
            coefb = P.tile([128, NT, 32], f32)
            cbuf = P.tile([128, NT, 26], bf16)
            nc.vector.memset(cbuf[:], 0)

            # =============== stages A+B: cv1 + cv2, chunk-pipelined ===============
            # B(k) is emitted one chunk behind A(k+1) so the PE never waits on
            # A's SiLU (scalar).  The 1x1 cv2 needs exactly A's chunk range.
            with tc.tile_pool(name="psA", bufs=2, space="PSUM") as psA, \
                 tc.tile_pool(name="psB", bufs=2, space="PSUM") as psB:

                def emit_b(t0, nn):
                    for m in range(2):
                        ps = psB.tile([128, 512], f32, tag="pb")
                        nc.tensor.matmul(ps[:, :nn], w2t[:, 128 * m:128 * m + 128],
                                         y1[:, t0:t0 + nn], start=True, stop=True)
                        nc.scalar.activation(y[:, m, YL + t0:YL + t0 + nn], ps[:, :nn],
                                             AF.Silu, bias=t2[:, m:m + 1], scale=s2[:, m:m + 1])

                prev = None
                t0 = 1
                while t0 < NPY - 1:
                    nn = min(512, NPY - 1 - t0)
                    ps = psA.tile([128, 512], f32, tag="pa")
                    first = True
                    for ck in range(2):
                        for s in range(9):
                            ki, kj = s // 3, s % 3
                            off = ki * W + kj - 1
                            nc.tensor.matmul(
                                ps[:, :nn], w1t[:, s, ck, :],
                                x_sb[:, ck, t0 + off: t0 + off + nn],
                                start=first, stop=(ck == 1 and s == 8))
                            first = False
                    nc.scalar.activation(y1[:, t0:t0 + nn], ps[:, :nn], AF.Silu,
                                         bias=t1[:], scale=s1[:])
                    if prev is not None:
                        emit_b(*prev)
                    prev = (t0, nn)
                    t0 += nn
                emit_b(*prev)
            # (y pixels 0 and NPY-1 are pad columns -> zeroed just below)
            y1pool.release()
            # zero lead/tail and pad columns; zero out-of-image rows via rowmask
            for m in range(2):
                zero_cast(y[:, m, 0:YL])
                zero_cast(y[:, m, YL + NPY:YW])
                yv = AP(y.tensor, y[:].offset + m * YW + 2, [[2 * YW, 128], [W, HY], [1, 2]])
                zero_cast(yv)
                yv2 = AP(y.tensor, y[:].offset + m * YW + W, [[2 * YW, 128], [W, HY], [1, 2]])
                zero_cast(yv2)
                nc.vector.tensor_scalar(y[:, m, YL:YL + 2 * W], y[:, m, YL:YL + 2 * W],
                                        rowm[:, 0:1], None, OP.mult)
                nc.vector.tensor_scalar(y[:, m, YL + NPY - 2 * W:YL + NPY],
                                        y[:, m, YL + NPY - 2 * W:YL + NPY],
                                        rowm[:, 1:2], None, OP.mult)

            # =============== stage D: depthwise conv + LN + GELU ===============
            # (stage C is emitted later, interleaved with D3/E, so its matmuls
            # fill the PE while the DVE does the LN normalize work)
            # D1: depthwise conv (diag matmuls) over the full padded range; per-pixel
            # channel sums / sumsq via N=1 matmuls (pixel-major stats on 128 lanes).
            statb = P.tile([128, NT, 2], f32)
            with tc.tile_pool(name="psD", bufs=3, space="PSUM") as psD, \
                 tc.tile_pool(name="psS", bufs=2, space="PSUM") as psS, \
                 tc.tile_pool(name="dtmp", bufs=2) as dtmp:
                t0 = 0
                while t0 < NTOT:
                    nn = min(512, NTOT - t0)
                    sc = nn // 128
                    for m in range(2):
                        ps = psD.tile([128, 512], f32, tag="pdw")
                        for ss in range(9):
                            ki, kj = ss // 3, ss % 3
                            off = (ki + 1) * W + kj - 1 + YL
                            nc.tensor.matmul(ps[:, :nn], dwdg[:, m, ss, :],
                                             y[:, m, t0 + off: t0 + off + nn],
                                             start=(ss == 0), stop=False)
                        nc.tensor.matmul(ps[:, :nn], dwb[:, m, :], ones_row[:, :nn],
                                         start=False, stop=True)
                        nc.vector.tensor_copy(dw[:, m, t0:t0 + nn], ps[:, :nn])
                        sqm = dtmp.tile([128, 512], bf16, tag=f"sq{m}")
                        nc.scalar.activation(sqm[:, :nn], ps[:, :nn], AF.Square)
                        if m == 0:
                            sq0 = sqm
                        else:
                            sq1 = sqm
                    pst = psS.tile([128, 8], f32, tag="pstat")
                    for sub in range(sc):
                        sl = slice(t0 + 128 * sub, t0 + 128 * sub + 128)
                        nc.tensor.matmul(pst[:, 2 * sub:2 * sub + 1], dw[:, 0, sl],
                                         ones_col[:], start=True, stop=False)
                        nc.tensor.matmul(pst[:, 2 * sub:2 * sub + 1], dw[:, 1, sl],
                                         ones_col[:], start=False, stop=True)
                        nc.tensor.matmul(pst[:, 2 * sub + 1:2 * sub + 2],
                                         sq0[:, 128 * sub:128 * sub + 128],
                                         ones_col[:], start=True, stop=False)
                        nc.tensor.matmul(pst[:, 2 * sub + 1:2 * sub + 2],
                                         sq1[:, 128 * sub:128 * sub + 128],
                                         ones_col[:], start=False, stop=True)
                    nc.vector.tensor_copy(statb[:, t0 // 128: t0 // 128 + sc, :], pst[:, :2 * sc])
                    t0 += nn

            # D2: stats math on [128, NT] (all lanes), then transpose into a
            # packed [NT, 256] tile: row T = [rstd (128 px) | brow (128 px)]
            tsb = P.tile([32, 256], bf16)
            with tc.tile_pool(name="stt", bufs=1) as sttp, \
                 tc.tile_pool(name="psST", bufs=2, space="PSUM") as psST:
                st0 = AP(statb.tensor, statb[:].offset, [[NT * 2, 128], [2, NT]])
                st1 = AP(statb.tensor, statb[:].offset + 1, [[NT * 2, 128], [2, NT]])
                meanb = sttp.tile([128, NT], f32)
                nc.vector.tensor_scalar(meanb[:], st0, 1.0 / 256, None, OP.mult)
                ex2 = sttp.tile([128, NT], f32)
                nc.vector.tensor_scalar(ex2[:], st1, 1.0 / 256, None, OP.mult)
                msq = sttp.tile([128, NT], f32)
                nc.scalar.activation(msq[:], meanb[:], AF.Square)
                nc.vector.tensor_tensor(ex2[:], ex2[:], msq[:], OP.subtract)
                sdev = sttp.tile([128, NT], f32)
                nc.scalar.activation(sdev[:], ex2[:], AF.Sqrt, bias=eps128[:], scale=1.0)
                rstdb = sttp.tile([128, NT], f32)
                with nc.allow_low_precision(reason="LN rstd"):
                    nc.vector.reciprocal(rstdb[:], sdev[:])
                browb = sttp.tile([128, NT], f32)
                nc.vector.scalar_tensor_tensor(browb[:], meanb[:], -1.0, rstdb[:],
                                               OP.mult, OP.mult)
                for ci, src in ((0, rstdb), (1, browb)):
                    pT = psST.tile([128, 128], f32, tag="pT")
                    nc.tensor.transpose(pT[:NT, :], src[:], ident[:])
                    nc.vector.tensor_copy(tsb[:NT, 128 * ci:128 * ci + 128], pT[:NT, :])

            # =============== stages D3 + C + E + coefficients + F, one pipeline ===============
            # D3 (LN normalize + gelu) interleaves with stage C chunks (PE
            # filler); then one full-size softmax/bilinear-prep pass (single
            # Exp table load); then per tile group the bilinear accumulation
            # (gpsimd mults + DVE strided adds) interleaves with stage F.
            GROUPS = [(0, 2), (2, 7), (7, 13), (13, 20), (20, 27)]

            pipe_ctx = ExitStack()
            with pipe_ctx:
                cfp = pipe_ctx.enter_context(tc.tile_pool(name="cf", bufs=1))
                mtp = pipe_ctx.enter_context(tc.tile_pool(name="mtp", bufs=2))
                msbp = pipe_ctx.enter_context(tc.tile_pool(name="msb", bufs=2))
                fin = pipe_ctx.enter_context(tc.tile_pool(name="fin", bufs=3))
                dtmp2 = pipe_ctx.enter_context(tc.tile_pool(name="dtmp2", bufs=3))
                de_ctx = ExitStack()
                psAB = de_ctx.enter_context(tc.tile_pool(name="psAB", bufs=2, space="PSUM"))
                psE = de_ctx.enter_context(tc.tile_pool(name="psE", bufs=2, space="PSUM"))
                psC = de_ctx.enter_context(tc.tile_pool(name="psC", bufs=2, space="PSUM"))

                def emit_c_chunk(Q):
                    p0 = 128 * Q
                    ps = psC.tile([128, 256], f32, tag="pc")
                    nc.tensor.matmul(ps[:], y[:, 0, p0:p0 + 128], wct[:, 0, :],
                                     start=True, stop=False)
                    nc.tensor.matmul(ps[:], y[:, 1, p0:p0 + 128], wct[:, 1, :],
                                     start=False, stop=False)
                    nc.tensor.matmul(ps[:], ones_row[:, 0:128], bcr[:],
                                     start=False, stop=True)
                    nc.vector.tensor_scalar(xpm[:, Q, :], ps[:],
                                            vmask[:, Q:Q + 1], None, OP.mult)

                def emit_d3_pair(T0, npair):
                    # rstd/brow broadcast via sel-matrix matmul
                    # (ps[i, j] = sum_k sel[k, T*128+i] tsb[k, j] = tsb[T, j])
                    t0 = 128 * T0
                    nn = 128 * npair
                    ps = psAB.tile([128, 512], f32, tag="pab")
                    for i in range(npair):
                        nc.tensor.matmul(ps[:, 256 * i:256 * i + 256],
                                         selm[:NT, t0 + 128 * i:t0 + 128 * (i + 1)],
                                         tsb[:NT, :], start=True, stop=True)
                    pa = AP(ps.tensor, ps[:].offset, [[512, 128], [0, 2], [256, npair], [1, 128]])
                    pb = AP(ps.tensor, ps[:].offset + 128, [[512, 128], [0, 2], [256, npair], [1, 128]])
                    zt = dtmp2.tile([128, 512], f32, tag="zt")
                    zv = AP(zt.tensor, zt[:].offset, [[512, 128], [256, 2], [128, npair], [1, 128]])
                    dwv = AP(dw.tensor, dw[:].offset + t0,
                             [[2 * NTOT, 128], [NTOT, 2], [128, npair], [1, 128]])
                    nc.vector.tensor_tensor(zv, dwv, pa, OP.mult)
                    nc.vector.tensor_tensor(zv, zv, pb, OP.add)
                    for m in range(2):
                        nc.scalar.activation(dw[:, m, t0:t0 + nn], zt[:, 256 * m:256 * m + nn],
                                             AF.Gelu, bias=lnb[:, m:m + 1], scale=lng[:, m:m + 1])

                def emit_e_tile(T):
                    ps = psE.tile([128, 32], f32, tag="pe")
                    nc.tensor.matmul(ps[:], dw[:, 0, 128 * T:128 * T + 128], womt[:, 0, :],
                                     start=True, stop=False)
                    nc.tensor.matmul(ps[:], dw[:, 1, 128 * T:128 * T + 128], womt[:, 1, :],
                                     start=False, stop=True)
                    nc.vector.tensor_tensor(coefb[:, T, :], ps[:], ombb[:], OP.add)

                cf_t = {}

                def emit_cf_phase1():
                    # softmax over masks + bilinear weight prep, full NT width
                    # (one Exp table load, big DVE ops)
                    cf = coefb[:].offset
                    cten = coefb.tensor

                    def cview(col0, step, cnt=9):
                        return AP(cten, cf + col0, [[NT * 32, 128], [32, NT], [step, cnt]])

                    ox = cview(0, 2)
                    oy = cview(1, 2)
                    lg = cview(18, 1)
                    t = cf_t
                    for nm in ("msm", "ix", "iy", "lx", "ly", "wx0", "wy0", "mx0",
                               "my0", "ta", "tb", "p00", "p01", "p10", "p11",
                               "gt", "contrib0", "contrib1"):
                        t[nm] = cfp.tile([128, NT, 9], bf16, tag=nm, name=nm)
                    mx = cfp.tile([128, NT], bf16, tag="mx")
                    nc.vector.tensor_reduce(mx[:], lg, axis=AX.X, op=OP.max)
                    E = cfp.tile([128, NT, 9], bf16, tag="E")
                    mxb = AP(mx.tensor, mx[:].offset, [[NT, 128], [1, NT], [0, 9]])
                    nc.vector.tensor_tensor(E[:], lg, mxb, OP.subtract)
                    nc.scalar.activation(E[:], E[:], AF.Exp)
                    se = cfp.tile([128, NT], bf16, tag="se")
                    with nc.allow_low_precision(reason="bf16 softmax"):
                        nc.vector.tensor_reduce(se[:], E[:], axis=AX.X, op=OP.add)
                        rs = cfp.tile([128, NT], bf16, tag="rs")
                        nc.vector.reciprocal(rs[:], se[:])
                    rsb = AP(rs.tensor, rs[:].offset, [[NT, 128], [1, NT], [0, 9]])
                    nc.vector.tensor_tensor(t["msm"][:], E[:], rsb, OP.mult)
                    # fractional parts and floor indicators
                    nc.vector.tensor_scalar(t["ix"][:], ox, 0.0, None, OP.is_lt)
                    nc.vector.tensor_scalar(t["iy"][:], oy, 0.0, None, OP.is_lt)
                    nc.vector.tensor_tensor(t["lx"][:], ox, t["ix"][:], OP.add)
                    nc.vector.tensor_tensor(t["ly"][:], oy, t["iy"][:], OP.add)
                    nc.vector.tensor_scalar(t["wx0"][:], t["lx"][:], -1.0, 1.0, OP.mult, OP.add)
                    nc.vector.tensor_scalar(t["wy0"][:], t["ly"][:], -1.0, 1.0, OP.mult, OP.add)
                    nc.vector.tensor_scalar(t["mx0"][:], t["ix"][:], -1.0, 1.0, OP.mult, OP.add)
                    nc.vector.tensor_scalar(t["my0"][:], t["iy"][:], -1.0, 1.0, OP.mult, OP.add)
                    nc.vector.tensor_tensor(t["ta"][:], t["msm"][:], t["wy0"][:], OP.mult)
                    nc.vector.tensor_tensor(t["tb"][:], t["msm"][:], t["ly"][:], OP.mult)
                    for a, tv in ((0, "ta"), (1, "tb")):
                        for b, wv in ((0, "wx0"), (1, "lx")):
                            nc.vector.tensor_tensor(t[f"p{a}{b}"][:], t[tv][:],
                                                    t[wv][:], OP.mult)

                def emit_cf_group(g0, GRP):
                    # bilinear 5x5 accumulation for tiles [g0, g0+GRP):
                    # contiguous mults on gpsimd, strided adds on DVE
                    t = cf_t

                    def sl(nm):
                        return t[nm][:, g0:g0 + GRP, :]

                    k = 0
                    for sy, myv in ((0, "my0"), (1, "iy")):
                        for sx, mxv in ((0, "mx0"), (1, "ix")):
                            nc.vector.tensor_tensor(sl("gt"), sl(myv), sl(mxv), OP.mult)
                            for a, b in ((0, 0), (0, 1), (1, 0), (1, 1)):
                                u0 = 1 + a - sy
                                v0 = 1 + b - sx
                                cb = t[f"contrib{k % 2}"]
                                k += 1
                                nc.vector.tensor_tensor(cb[:, g0:g0 + GRP, :],
                                                        sl(f"p{a}{b}"), sl("gt"), OP.mult)
                                # C5[:, :, u0 + j, v0 + i] += contrib[i, j]
                                dstv = AP(cbuf.tensor,
                                          cbuf[:].offset + 26 * g0 + (u0 * 5 + v0),
                                          [[NT * 26, 128], [26, GRP], [1, 3], [5, 3]])
                                srcv = AP(cb.tensor, cb[:].offset + 9 * g0,
                                          [[NT * 9, 128], [9, GRP], [3, 3], [1, 3]])
                                nc.vector.tensor_tensor(dstv, dstv, srcv, OP.add)

                def emit_f_tile(T, psT, psZ):
                    mt = mtp.tile([128, MTW], bf16, tag="mt")
                    nc.gpsimd.local_scatter(mt[:], cbuf[:, T, :],
                                            sidx[:], channels=128, num_elems=MTW,
                                            num_idxs=26)
                    msb = msbp.tile([128, NCI, 128], bf16, tag="msb")
                    for cp in range(NCI // 2):
                        pst = psT.tile([128, 256], bf16, tag="pst")
                        for j in range(2):
                            ci = 2 * cp + j
                            nc.tensor.transpose(pst[:, 128 * j:128 * j + 128],
                                                mt[:, 128 * ci:128 * ci + 128], identb[:])
                        if cp == 0:
                            nc.vector.tensor_copy(msb[:, 2 * cp:2 * cp + 2, :], pst[:])
                        else:
                            nc.scalar.copy(msb[:, 2 * cp:2 * cp + 2, :], pst[:])
                    psz = psZ.tile([128, 256], f32, tag="psz")
                    for ci in range(NCI):
                        nc.tensor.matmul(psz[:], msb[:, ci, :], xpm[:, T + ci, :],
                                         start=(ci == 0), stop=False)
                    nc.tensor.matmul(psz[:], ones_row[:, 0:128], t3r[:],
                                     start=False, stop=True)
                    xrt = fin.tile([128, 256], bf16, tag="xrt")
                    nc.sync.dma_start(xrt[:], d_xr.ap()[T])
                    zact = fin.tile([128, 256], f32, tag="zact")
                    nc.scalar.activation(zact[:], psz[:], AF.Silu)
                    osb = fin.tile([128, 256], bf16, tag="osb")
                    nc.vector.tensor_tensor(osb[:], zact[:], xrt[:], OP.add)
                    nc.sync.dma_start(d_out.ap()[T], osb[:])

                # D3 pairs interleaved with C chunks and E tiles (C's matmuls
                # keep the PE busy while the DVE normalizes)
                ptr_c = 0
                for T0 in range(0, NT, 2):
                    npair = min(2, NT - T0)
                    emit_d3_pair(T0, npair)
                    for _ in range(2):
                        if ptr_c < 24:
                            emit_c_chunk(ptr_c)
                            ptr_c += 1
                    for T in range(T0, T0 + npair):
                        emit_e_tile(T)

                emit_cf_phase1()
                # remaining Xpm chunks: PE filler under phase1's DVE work
                while ptr_c < XCH:
                    emit_c_chunk(ptr_c)
                    ptr_c += 1
                de_ctx.close()
                with tc.tile_pool(name="psT", bufs=2, space="PSUM") as psT, \
                     tc.tile_pool(name="psZ", bufs=3, space="PSUM") as psZ:
                    for ga, gb in GROUPS:
                        emit_cf_group(ga, gb - ga)
                        for T in range(ga, gb):
                            emit_f_tile(T, psT, psZ)

            if dump:
                nc.sync.dma_start(d_dy1.ap(), y1[:])
                nc.sync.dma_start(d_dy.ap(), y[:].rearrange("p a b -> p (a b)"))
                nc.sync.dma_start(d_ddwg.ap(), dw[:].rearrange("p a b -> p (a b)"))
                nc.sync.dma_start(d_dxpm.ap(), xpm[:].rearrange("p a b -> p (a b)"))
                nc.sync.dma_start(d_dcoef.ap(), coefb[:].rearrange("p a b -> p (a b)"))
                nc.sync.dma_start(d_dcbuf.ap(), cbuf[:].rearrange("p a b -> p (a b)"))

    nc.compile()
    return nc


def _get_built():
    global _BUILT
    if _BUILT is None:
        _BUILT = _build()
    return _BUILT


def _bf(a):
    return np.asarray(a, dtype=ml_dtypes.bfloat16)


def _prep(inputs):
    g = {k: np.asarray(v, dtype=np.float32) for k, v in inputs.items()}
    x = g["x"]

    s1 = g["g1"] / np.sqrt(g["v1"] + EPS_BN)
    t1 = g["b1"] - g["m1"] * s1
    s2 = g["g2"] / np.sqrt(g["v2"] + EPS_BN)
    t2 = g["b2"] - g["m2"] * s2
    s3 = g["g3"] / np.sqrt(g["v3"] + EPS_BN)
    t3 = g["b3"] - g["m3"] * s3

    w1 = g["w1"]  # [128, 256, 3, 3]
    w1t = np.zeros((9, 2, 128, 128), np.float32)
    for ki in range(3):
        for kj in range(3):
            for ck in range(2):
                w1t[ki * 3 + kj, ck] = w1[:, 128 * ck:128 * ck + 128, ki, kj].T
    w2t = g["w2"][:, :, 0, 0].T.copy()  # [128, 256]
    Wc = g["out_w"] @ g["in_w"]
    wct = np.stack([Wc.T[:128], Wc.T[128:]])  # [2, 128, 256]
    bc = (g["out_w"] @ g["in_b"] + g["out_b"])[None, :]  # [1, 256]
    dwdg = np.zeros((2, 9, 128, 128), np.float32)
    for ck in range(2):
        for s in range(9):
            np.fill_diagonal(dwdg[ck, s], g["dw_w"][128 * ck:128 * ck + 128, 0, s // 3, s % 3])
    dwb = np.zeros((2, 1, 128), np.float32)
    dwb[0, 0] = g["dw_b"][:128]
    dwb[1, 0] = g["dw_b"][128:]
    womt = np.zeros((2, 128, 32), np.float32)
    for ck in range(2):
        womt[ck, :, :18] = g["off_w"][:, 128 * ck:128 * ck + 128].T
        womt[ck, :, 18:27] = g["msk_w"][:, 128 * ck:128 * ck + 128].T
    ombb = np.zeros((128, 32), np.float32)
    ombb[:, :18] = g["off_b"][None, :]
    ombb[:, 18:27] = g["msk_b"][None, :]
    ident = np.eye(128, dtype=np.float32)

    def colsplit(v):  # [256] -> [2, 128, 1]
        return v.reshape(2, 128, 1).astype(np.float32)

    # fold BN3 scale into the combined projection; t3 added on-chip
    wct = (wct.reshape(2, 128, 256) * s3[None, None, :]).astype(np.float32)
    bc = (bc * s3[None, :]).astype(np.float32)

    selm = np.zeros((32, NT * 128), np.float32)
    for T in range(NT):
        selm[T, 128 * T:128 * T + 128] = 1.0

    shared = dict(
        zeros=np.zeros((128, 512), np.float32),
        onesc=_bf(np.ones((128, 1))),
        onesr=_bf(np.ones((1, 512))),
        selm=_bf(selm),
        w1t=_bf(w1t.transpose(2, 0, 1, 3).copy()), w2t=_bf(w2t),
        wct=_bf(wct.transpose(1, 0, 2).copy()), bcr=_bf(bc),
        dwdg=_bf(dwdg.transpose(2, 0, 1, 3).copy()), dwb=_bf(dwb),
        womt=_bf(womt), ombb=ombb,
        s1=s1[:, None], t1=t1[:, None],
        s2=colsplit(s2), t2=colsplit(t2),
        lng=colsplit(g["ln_g"]), lnb=colsplit(g["ln_b"]),
        ident=ident, identb=_bf(ident),
        t3r=_bf(t3[None, :]),
    )

    in_maps = []
    for c in range(8):
        n, h = c // 2, c % 2
        r0 = 40 * h - 3  # x frame row 0 in global coords
        xs = np.zeros((2, 128, HX, W), np.float32)
        glo = max(r0, 0)
        ghi = min(r0 + HX, 80)
        xs[0, :, glo - r0:ghi - r0, 2:82] = x[n, :128, glo:ghi, :]
        xs[1, :, glo - r0:ghi - r0, 2:82] = x[n, 128:, glo:ghi, :]
        # validity mask for Xpm pixels: q = 128*Q + p -> y-pix q - YL
        vm = np.zeros((XCH * 128,), np.float32)
        qs = np.arange(XCH * 128)
        pix = qs - YL
        rv, cv = pix // W, pix % W
        gr = 40 * h + rv - 2
        ok = (pix >= 0) & (pix < NPY) & (cv >= 2) & (cv < 82) & (gr >= 0) & (gr < 80)
        vm[ok] = 1.0
        vmask = vm.reshape(XCH, 128).T.copy()  # [128, XCH]
        rowm = np.zeros((2, 128, 1), np.float32)
        rowm[0] = 0.0 if h == 0 else 1.0   # y rows [0,2) valid only for h=1
        rowm[1] = 1.0 if h == 0 else 0.0   # y rows [42,44) valid only for h=0
        # pixel-major residual input: xr[T, p, c] = x at out-frame pixel 128T+p
        xflat = np.concatenate([xs[0], xs[1]], 0).reshape(256, HX, W)
        xres = xflat[:, 3:43, :].reshape(256, NPD).T  # [NPD, 256]
        xr = np.zeros((NT * 128, 256), np.float32)
        xr[:NPD] = xres
        m = dict(shared)
        m["xr"] = _bf(xr.reshape(NT, 128, 256))
        m["xs"] = _bf(xs.reshape(2, 128, 4, NPX // 4).transpose(2, 1, 0, 3).copy())
        m["vmask"] = vmask
        m["rowm"] = rowm
        in_maps.append(m)
    return in_maps


def kernel(**inputs):
    nc = _get_built()
    in_maps = _prep(inputs)
    res = bass_utils.run_bass_kernel_spmd(nc, in_maps, core_ids=list(range(8)))
    out = np.zeros((4, 256, 80, 80), np.float32)
    for c in range(8):
        n, h = c // 2, c % 2
        o = np.asarray(res.results[c]["out"], np.float32).reshape(NT * 128, 256)[:NPD]
        o = o.reshape(HD, W, 256)[:, 2:82].transpose(2, 0, 1)
        out[n, :, 40 * h:40 * h + 40, :] = o
    return out


# revision 57
# speedup vs baseline: 1.0657x; 1.0411x over previous
"""Trainium2 Bass kernel for the DCNv4 bottleneck block.

Contract: kernel(**inputs) takes FULL unsharded inputs (as in reference
setup_inputs()) and returns the FULL (4, 256, 80, 80) fp32 output.

Sharding: 8 cores = 4 samples x 2 row-halves (40 rows each + halos).

Per-core pipeline (channel-major [C-part, flat-pixel] on an 84-wide frame,
all matmul operands bf16, fp32 PSUM accumulation):
  cv1 3x3 conv (9 shifted matmuls) + BN + SiLU               -> y1 [128, 3696]
  cv2 1x1 conv + BN + SiLU                                   -> y  [2][128, 3840]
  depthwise 3x3 (diag matmuls) + LayerNorm + GELU            -> dw  [2][128, 3456]
  combined in/out projection Xc = (out_w@in_w) y + bias      -> Xpm pixel-major [128, 30, 256]
  offset/mask projection (pixel-major) -> bilinear/mask coefficients
  deformable sampling as banded matmul: M^T built by GPSIMD local_scatter
  (bf16), PE-transposed to M chunks, out[t, c] = sum_q M[q, t] Xpm[q, c]
  BN3 + SiLU + residual, store channel-major.

The y buffer keeps a 2-column lead so Xpm chunk Q is exactly y columns
[128Q, 128Q+128): each out tile T samples q in [128T, 128T+468) -> 4 chunks.
"""

import numpy as np
import ml_dtypes
from contextlib import ExitStack

import concourse.bass as bass
import concourse.tile as tile
from concourse import bacc, mybir
from concourse import bass_utils
from concourse.ap import AP

f32 = mybir.dt.float32
bf16 = mybir.dt.bfloat16
i16 = mybir.dt.int16
AF = mybir.ActivationFunctionType
OP = mybir.AluOpType
AX = mybir.AxisListType

# ---- geometry constants ----
W = 84                  # frame width (80 image + 2 pad each side)
HX, HY, HD = 46, 44, 40
NPX = HX * W            # 3864  x frame pixels
NPY = HY * W            # 3696  y frame pixels
YL = 2                  # y buffer lead columns (y-pix p stored at col p+2)
YW = 3840               # y buffer width = XCH*128
NPD = HD * W            # 3360  out-region pixels
NTOT = 3456             # padded out pixels (27 tiles)
NT = 27                 # out-pixel tiles of 128
XCH = 30                # Xpm chunks of 128 (xpm q == y column q)
NCI = 4                 # M chunks per out tile (band q - t in [0, 468))
MTW = 512               # M^T row width
EPS_BN, EPS_LN = 1e-5, 1e-6

_BUILT = None  # cached (nc,)


def _build(dump=False):
    nc = bacc.Bacc("TRN2", target_bir_lowering=False, debug=False, num_devices=8)

    # ---------------- DRAM I/O ----------------
    d_x = nc.dram_tensor("xs", [4, 128, 2, NPX // 4], bf16, kind="ExternalInput")
    d_w1t = nc.dram_tensor("w1t", [128, 9, 2, 128], bf16, kind="ExternalInput")
    d_w2t = nc.dram_tensor("w2t", [128, 256], bf16, kind="ExternalInput")
    d_wct = nc.dram_tensor("wct", [128, 2, 256], bf16, kind="ExternalInput")
    d_bc = nc.dram_tensor("bcr", [1, 256], bf16, kind="ExternalInput")
    d_dwdg = nc.dram_tensor("dwdg", [128, 2, 9, 128], bf16, kind="ExternalInput")
    d_dwb = nc.dram_tensor("dwb", [2, 1, 128], bf16, kind="ExternalInput")
    d_womt = nc.dram_tensor("womt", [2, 128, 32], bf16, kind="ExternalInput")
    d_ombb = nc.dram_tensor("ombb", [128, 32], f32, kind="ExternalInput")
    d_s1 = nc.dram_tensor("s1", [128, 1], f32, kind="ExternalInput")
    d_t1 = nc.dram_tensor("t1", [128, 1], f32, kind="ExternalInput")
    d_s2 = nc.dram_tensor("s2", [2, 128, 1], f32, kind="ExternalInput")
    d_t2 = nc.dram_tensor("t2", [2, 128, 1], f32, kind="ExternalInput")
    d_lng = nc.dram_tensor("lng", [2, 128, 1], f32, kind="ExternalInput")
    d_lnb = nc.dram_tensor("lnb", [2, 128, 1], f32, kind="ExternalInput")
    d_ident = nc.dram_tensor("ident", [128, 128], f32, kind="ExternalInput")
    d_identb = nc.dram_tensor("identb", [128, 128], bf16, kind="ExternalInput")
    d_vmask = nc.dram_tensor("vmask", [128, XCH], f32, kind="ExternalInput")
    d_rowm = nc.dram_tensor("rowm", [2, 128, 1], f32, kind="ExternalInput")
    d_zeros = nc.dram_tensor("zeros", [128, 512], f32, kind="ExternalInput")
    d_xr = nc.dram_tensor("xr", [NT, 128, 256], bf16, kind="ExternalInput")
    d_t3r = nc.dram_tensor("t3r", [1, 256], bf16, kind="ExternalInput")
    d_onesc = nc.dram_tensor("onesc", [128, 1], bf16, kind="ExternalInput")
    d_onesr = nc.dram_tensor("onesr", [1, 512], bf16, kind="ExternalInput")
    d_selm = nc.dram_tensor("selm", [32, NT * 128], bf16, kind="ExternalInput")
    d_out = nc.dram_tensor("out", [NT, 128, 256], bf16, kind="ExternalOutput")
    if dump:
        d_dy1 = nc.dram_tensor("dy1", [128, NPY], bf16, kind="ExternalOutput")
        d_dy = nc.dram_tensor("dy", [128, 2 * YW], bf16, kind="ExternalOutput")
        d_ddwg = nc.dram_tensor("ddwg", [128, 2 * NTOT], bf16, kind="ExternalOutput")
        d_dxpm = nc.dram_tensor("dxpm", [128, XCH * 256], bf16, kind="ExternalOutput")
        d_dcoef = nc.dram_tensor("dcoef", [128, NT * 32], f32, kind="ExternalOutput")
        d_dcbuf = nc.dram_tensor("dcbuf", [128, NT * 26], bf16, kind="ExternalOutput")

    with tile.TileContext(nc) as tc:
        with ExitStack() as ctx:
            P = ctx.enter_context(tc.tile_pool(name="persist", bufs=1))

            # ---------------- loads ----------------
            # big A-stage inputs on the gpsimd DMA queue (first to arrive);
            # everything else spread over the vector/scalar/sync queues
            x_sb = P.tile([128, 2, NPX], bf16)
            w1t = P.tile([128, 9, 2, 128], bf16)
            nc.gpsimd.dma_start(w1t[:], d_w1t.ap())
            SEG = NPX // 4
            for i in range(4):
                nc.gpsimd.dma_start(x_sb[:, :, SEG * i:SEG * (i + 1)], d_x.ap()[i])
            w2t = P.tile([128, 256], bf16)
            nc.scalar.dma_start(w2t[:], d_w2t.ap())
            wct = P.tile([128, 2, 256], bf16)
            nc.scalar.dma_start(wct[:], d_wct.ap())
            bcr = P.tile([1, 256], bf16)
            nc.scalar.dma_start(bcr[:], d_bc.ap())
            dwdg = P.tile([128, 2, 9, 128], bf16)
            nc.scalar.dma_start(dwdg[:], d_dwdg.ap())
            dwb = P.tile([1, 2, 128], bf16)
            nc.scalar.dma_start(dwb[:], d_dwb.ap().transpose([1, 0, 2]))
            womt = P.tile([128, 2, 32], bf16)
            nc.scalar.dma_start(womt[:], d_womt.ap().transpose([1, 0, 2]))
            ombb = P.tile([128, 32], f32)
            nc.sync.dma_start(ombb[:], d_ombb.ap())
            s1 = P.tile([128, 1], f32)
            nc.sync.dma_start(s1[:], d_s1.ap())
            t1 = P.tile([128, 1], f32)
            nc.sync.dma_start(t1[:], d_t1.ap())
            s2 = P.tile([128, 2], f32)
            nc.sync.dma_start(s2[:], d_s2.ap().transpose([1, 0, 2]))
            t2 = P.tile([128, 2], f32)
            nc.sync.dma_start(t2[:], d_t2.ap().transpose([1, 0, 2]))
            lng = P.tile([128, 2], f32)
            nc.sync.dma_start(lng[:], d_lng.ap().transpose([1, 0, 2]))
            lnb = P.tile([128, 2], f32)
            nc.sync.dma_start(lnb[:], d_lnb.ap().transpose([1, 0, 2]))
            ident = P.tile([128, 128], f32)
            nc.sync.dma_start(ident[:], d_ident.ap())
            identb = P.tile([128, 128], bf16)
            nc.sync.dma_start(identb[:], d_identb.ap())
            vmask = P.tile([128, XCH], f32)
            nc.sync.dma_start(vmask[:], d_vmask.ap())
            rowm = P.tile([128, 2], f32)
            nc.sync.dma_start(rowm[:], d_rowm.ap().transpose([1, 0, 2]))
            t3r = P.tile([1, 256], bf16)
            nc.scalar.dma_start(t3r[:], d_t3r.ap())
            selm = P.tile([32, NT * 128], bf16)
            nc.scalar.dma_start(selm[:], d_selm.ap())

            zref = P.tile([128, 512], f32)
            nc.scalar.dma_start(zref[:], d_zeros.ap())

            def zero_cast(dst_ap):
                # DVE cast-copy zeros onto any view (verifier-clean)
                src = AP(zref.tensor, zref[:].offset,
                         [[512, dst_ap.ap[0][1]]] + [[0, d[1]] for d in dst_ap.ap[1:]])
                nc.vector.tensor_copy(dst_ap, src)

            ones_row = P.tile([1, 512], bf16)
            nc.gpsimd.dma_start(ones_row[:], d_onesr.ap())
            ones_col = P.tile([128, 1], bf16)
            nc.sync.dma_start(ones_col[:], d_onesc.ap())
            eps128 = P.tile([128, 1], f32)
            nc.vector.memset(eps128[:], EPS_LN)

            # scatter indices for M^T build: idx = t + 84u + v, u,v in [0,5)
            sidx = P.tile([128, 26], i16)
            nc.gpsimd.iota(sidx[:, 0:25], pattern=[[W, 5], [1, 5]], base=0,
                           channel_multiplier=1, allow_small_or_imprecise_dtypes=True)
            nc.vector.memset(sidx[:, 25:26], -1)

            # ---------------- persistent activations ----------------
            y1pool = tc.alloc_tile_pool(name="y1pool", bufs=1)
            y1 = y1pool.tile([128, NPY], bf16)
            y = P.tile([128, 2, YW], bf16)
            dw = P.tile([128, 2, NTOT], bf16)       # later overwritten by gelu output
            xpm = P.tile([128, XCH, 256], bf16)
            coefb = P.tile([128, NT, 32], f32)
            cbuf = P.tile([128, NT, 26], bf16)
            nc.vector.memset(cbuf[:], 0)

            # =============== stages A+B: cv1 + cv2, chunk-pipelined ===============
            # B(k) is emitted one chunk behind A(k+1) so the PE never waits on
            # A's SiLU (scalar).  The 1x1 cv2 needs exactly A's chunk range.
            with tc.tile_pool(name="psA", bufs=2, space="PSUM") as psA, \
                 tc.tile_pool(name="psB", bufs=2, space="PSUM") as psB:

                def emit_b(t0, nn):
                    for m in range(2):
                        ps = psB.tile([128, 512], f32, tag="pb")
                        nc.tensor.matmul(ps[:, :nn], w2t[:, 128 * m:128 * m + 128],
                                         y1[:, t0:t0 + nn], start=True, stop=True)
                        nc.scalar.activation(y[:, m, YL + t0:YL + t0 + nn], ps[:, :nn],
                                             AF.Silu, bias=t2[:, m:m + 1], scale=s2[:, m:m + 1])

                prev = None
                t0 = 1
                while t0 < NPY - 1:
                    nn = min(512, NPY - 1 - t0)
                    ps = psA.tile([128, 512], f32, tag="pa")
                    first = True
                    for ck in range(2):
                        for s in range(9):
                            ki, kj = s // 3, s % 3
                            off = ki * W + kj - 1
                            nc.tensor.matmul(
                                ps[:, :nn], w1t[:, s, ck, :],
                                x_sb[:, ck, t0 + off: t0 + off + nn],
                                start=first, stop=(ck == 1 and s == 8))
                            first = False
                    nc.scalar.activation(y1[:, t0:t0 + nn], ps[:, :nn], AF.Silu,
                                         bias=t1[:], scale=s1[:])
                    if prev is not None:
                        emit_b(*prev)
                    prev = (t0, nn)
                    t0 += nn
                emit_b(*prev)
            # (y pixels 0 and NPY-1 are pad columns -> zeroed just below)
            y1pool.release()
            # zero lead/tail and pad columns; zero out-of-image rows via rowmask
            for m in range(2):
                zero_cast(y[:, m, 0:YL])
                zero_cast(y[:, m, YL + NPY:YW])
                yv = AP(y.tensor, y[:].offset + m * YW + 2, [[2 * YW, 128], [W, HY], [1, 2]])
                zero_cast(yv)
                yv2 = AP(y.tensor, y[:].offset + m * YW + W, [[2 * YW, 128], [W, HY], [1, 2]])
                zero_cast(yv2)
                nc.vector.tensor_scalar(y[:, m, YL:YL + 2 * W], y[:, m, YL:YL + 2 * W],
                                        rowm[:, 0:1], None, OP.mult)
                nc.vector.tensor_scalar(y[:, m, YL + NPY - 2 * W:YL + NPY],
                                        y[:, m, YL + NPY - 2 * W:YL + NPY],
                                        rowm[:, 1:2], None, OP.mult)

            # =============== stage D: depthwise conv + LN + GELU ===============
            # (stage C is emitted later, interleaved with D3/E, so its matmuls
            # fill the PE while the DVE does the LN normalize work)
            # D1: depthwise conv (diag matmuls) over the full padded range; per-pixel
            # channel sums / sumsq via N=1 matmuls (pixel-major stats on 128 lanes).
            statb = P.tile([128, NT, 2], f32)
            with tc.tile_pool(name="psD", bufs=3, space="PSUM") as psD, \
                 tc.tile_pool(name="psS", bufs=2, space="PSUM") as psS, \
                 tc.tile_pool(name="dtmp", bufs=2) as dtmp:
                t0 = 0
                while t0 < NTOT:
                    nn = min(512, NTOT - t0)
                    sc = nn // 128
                    for m in range(2):
                        ps = psD.tile([128, 512], f32, tag="pdw")
                        for ss in range(9):
                            ki, kj = ss // 3, ss % 3
                            off = (ki + 1) * W + kj - 1 + YL
                            nc.tensor.matmul(ps[:, :nn], dwdg[:, m, ss, :],
                                             y[:, m, t0 + off: t0 + off + nn],
                                             start=(ss == 0), stop=False)
                        nc.tensor.matmul(ps[:, :nn], dwb[:, m, :], ones_row[:, :nn],
                                         start=False, stop=True)
                        nc.vector.tensor_copy(dw[:, m, t0:t0 + nn], ps[:, :nn])
                        sqm = dtmp.tile([128, 512], bf16, tag=f"sq{m}")
                        nc.scalar.activation(sqm[:, :nn], ps[:, :nn], AF.Square)
                        if m == 0:
                            sq0 = sqm
                        else:
                            sq1 = sqm
                    pst = psS.tile([128, 8], f32, tag="pstat")
                    for sub in range(sc):
                        sl = slice(t0 + 128 * sub, t0 + 128 * sub + 128)
                        nc.tensor.matmul(pst[:, 2 * sub:2 * sub + 1], dw[:, 0, sl],
                                         ones_col[:], start=True, stop=False)
                        nc.tensor.matmul(pst[:, 2 * sub:2 * sub + 1], dw[:, 1, sl],
                                         ones_col[:], start=False, stop=True)
                        nc.tensor.matmul(pst[:, 2 * sub + 1:2 * sub + 2],
                                         sq0[:, 128 * sub:128 * sub + 128],
                                         ones_col[:], start=True, stop=False)
                        nc.tensor.matmul(pst[:, 2 * sub + 1:2 * sub + 2],
                                         sq1[:, 128 * sub:128 * sub + 128],
                                         ones_col[:], start=False, stop=True)
                    nc.vector.tensor_copy(statb[:, t0 // 128: t0 // 128 + sc, :], pst[:, :2 * sc])
                    t0 += nn

            # D2: stats math on [128, NT] (all lanes), then transpose into a
            # packed [NT, 256] tile: row T = [rstd (128 px) | brow (128 px)]
            tsb = P.tile([32, 256], bf16)
            with tc.tile_pool(name="stt", bufs=1) as sttp, \
                 tc.tile_pool(name="psST", bufs=2, space="PSUM") as psST:
                st0 = AP(statb.tensor, statb[:].offset, [[NT * 2, 128], [2, NT]])
                st1 = AP(statb.tensor, statb[:].offset + 1, [[NT * 2, 128], [2, NT]])
                meanb = sttp.tile([128, NT], f32)
                nc.vector.tensor_scalar(meanb[:], st0, 1.0 / 256, None, OP.mult)
                ex2 = sttp.tile([128, NT], f32)
                nc.vector.tensor_scalar(ex2[:], st1, 1.0 / 256, None, OP.mult)
                msq = sttp.tile([128, NT], f32)
                nc.scalar.activation(msq[:], meanb[:], AF.Square)
                nc.vector.tensor_tensor(ex2[:], ex2[:], msq[:], OP.subtract)
                sdev = sttp.tile([128, NT], f32)
                nc.scalar.activation(sdev[:], ex2[:], AF.Sqrt, bias=eps128[:], scale=1.0)
                rstdb = sttp.tile([128, NT], f32)
                with nc.allow_low_precision(reason="LN rstd"):
                    nc.vector.reciprocal(rstdb[:], sdev[:])
                browb = sttp.tile([128, NT], f32)
                nc.vector.scalar_tensor_tensor(browb[:], meanb[:], -1.0, rstdb[:],
                                               OP.mult, OP.mult)
                for ci, src in ((0, rstdb), (1, browb)):
                    pT = psST.tile([128, 128], f32, tag="pT")
                    nc.tensor.transpose(pT[:NT, :], src[:], ident[:])
                    nc.vector.tensor_copy(tsb[:NT, 128 * ci:128 * ci + 128], pT[:NT, :])

            # =============== stages D3 + C + E + coefficients + F, one pipeline ===============
            # D3 (LN normalize + gelu) interleaves with stage C chunks (PE
            # filler); then one full-size softmax/bilinear-prep pass (single
            # Exp table load); then per tile group the bilinear accumulation
            # (gpsimd mults + DVE strided adds) interleaves with stage F.
            GROUPS = [(0, 2), (2, 7), (7, 13), (13, 20), (20, 27)]

            pipe_ctx = ExitStack()
            with pipe_ctx:
                cfp = pipe_ctx.enter_context(tc.tile_pool(name="cf", bufs=1))
                mtp = pipe_ctx.enter_context(tc.tile_pool(name="mtp", bufs=2))
                msbp = pipe_ctx.enter_context(tc.tile_pool(name="msb", bufs=2))
                fin = pipe_ctx.enter_context(tc.tile_pool(name="fin", bufs=3))
                dtmp2 = pipe_ctx.enter_context(tc.tile_pool(name="dtmp2", bufs=3))
                de_ctx = ExitStack()
                psAB = de_ctx.enter_context(tc.tile_pool(name="psAB", bufs=2, space="PSUM"))
                psE = de_ctx.enter_context(tc.tile_pool(name="psE", bufs=2, space="PSUM"))
                psC = de_ctx.enter_context(tc.tile_pool(name="psC", bufs=2, space="PSUM"))

                def emit_c_chunk(Q):
                    p0 = 128 * Q
                    ps = psC.tile([128, 256], f32, tag="pc")
                    nc.tensor.matmul(ps[:], y[:, 0, p0:p0 + 128], wct[:, 0, :],
                                     start=True, stop=False)
                    nc.tensor.matmul(ps[:], y[:, 1, p0:p0 + 128], wct[:, 1, :],
                                     start=False, stop=False)
                    nc.tensor.matmul(ps[:], ones_row[:, 0:128], bcr[:],
                                     start=False, stop=True)
                    if Q % 2 == 0:
                        nc.vector.tensor_scalar(xpm[:, Q, :], ps[:],
                                                vmask[:, Q:Q + 1], None, OP.mult)
                    else:
                        nc.scalar.mul(xpm[:, Q, :], ps[:], vmask[:, Q:Q + 1])

                def emit_d3_pair(T0, npair):
                    # rstd/brow broadcast via sel-matrix matmul
                    # (ps[i, j] = sum_k sel[k, T*128+i] tsb[k, j] = tsb[T, j])
                    t0 = 128 * T0
                    nn = 128 * npair
                    ps = psAB.tile([128, 512], f32, tag="pab")
                    for i in range(npair):
                        nc.tensor.matmul(ps[:, 256 * i:256 * i + 256],
                                         selm[:NT, t0 + 128 * i:t0 + 128 * (i + 1)],
                                         tsb[:NT, :], start=True, stop=True)
                    pa = AP(ps.tensor, ps[:].offset, [[512, 128], [0, 2], [256, npair], [1, 128]])
                    pb = AP(ps.tensor, ps[:].offset + 128, [[512, 128], [0, 2], [256, npair], [1, 128]])
                    zt = dtmp2.tile([128, 512], f32, tag="zt")
                    zv = AP(zt.tensor, zt[:].offset, [[512, 128], [256, 2], [128, npair], [1, 128]])
                    dwv = AP(dw.tensor, dw[:].offset + t0,
                             [[2 * NTOT, 128], [NTOT, 2], [128, npair], [1, 128]])
                    nc.vector.tensor_tensor(zv, dwv, pa, OP.mult)
                    nc.vector.tensor_tensor(zv, zv, pb, OP.add)
                    for m in range(2):
                        nc.scalar.activation(dw[:, m, t0:t0 + nn], zt[:, 256 * m:256 * m + nn],
                                             AF.Gelu, bias=lnb[:, m:m + 1], scale=lng[:, m:m + 1])

                def emit_e_tile(T):
                    ps = psE.tile([128, 32], f32, tag="pe")
                    nc.tensor.matmul(ps[:], dw[:, 0, 128 * T:128 * T + 128], womt[:, 0, :],
                                     start=True, stop=False)
                    nc.tensor.matmul(ps[:], dw[:, 1, 128 * T:128 * T + 128], womt[:, 1, :],
                                     start=False, stop=True)
                    nc.vector.tensor_tensor(coefb[:, T, :], ps[:], ombb[:], OP.add)

                cf_t = {}

                def emit_cf_phase1():
                    # softmax over masks + bilinear weight prep, full NT width
                    # (one Exp table load, big DVE ops)
                    cf = coefb[:].offset
                    cten = coefb.tensor

                    def cview(col0, step, cnt=9):
                        return AP(cten, cf + col0, [[NT * 32, 128], [32, NT], [step, cnt]])

                    ox = cview(0, 2)
                    oy = cview(1, 2)
                    lg = cview(18, 1)
                    t = cf_t
                    for nm in ("msm", "ix", "iy", "lx", "ly", "wx0", "wy0", "mx0",
                               "my0", "ta", "tb", "Y0", "Y1", "Y2", "X0", "X1",
                               "X2", "tmp", "contrib0", "contrib1"):
                        t[nm] = cfp.tile([128, NT, 9], bf16, tag=nm, name=nm)
                    mx = cfp.tile([128, NT], bf16, tag="mx")
                    nc.vector.tensor_reduce(mx[:], lg, axis=AX.X, op=OP.max)
                    E = cfp.tile([128, NT, 9], bf16, tag="E")
                    mxb = AP(mx.tensor, mx[:].offset, [[NT, 128], [1, NT], [0, 9]])
                    nc.vector.tensor_tensor(E[:], lg, mxb, OP.subtract)
                    nc.scalar.activation(E[:], E[:], AF.Exp)
                    se = cfp.tile([128, NT], bf16, tag="se")
                    with nc.allow_low_precision(reason="bf16 softmax"):
                        nc.vector.tensor_reduce(se[:], E[:], axis=AX.X, op=OP.add)
                        rs = cfp.tile([128, NT], bf16, tag="rs")
                        nc.vector.reciprocal(rs[:], se[:])
                    rsb = AP(rs.tensor, rs[:].offset, [[NT, 128], [1, NT], [0, 9]])
                    nc.vector.tensor_tensor(t["msm"][:], E[:], rsb, OP.mult)
                    # fractional parts and floor indicators
                    nc.vector.tensor_scalar(t["ix"][:], ox, 0.0, None, OP.is_lt)
                    nc.vector.tensor_scalar(t["iy"][:], oy, 0.0, None, OP.is_lt)
                    nc.vector.tensor_tensor(t["lx"][:], ox, t["ix"][:], OP.add)
                    nc.vector.tensor_tensor(t["ly"][:], oy, t["iy"][:], OP.add)
                    nc.vector.tensor_scalar(t["wx0"][:], t["lx"][:], -1.0, 1.0, OP.mult, OP.add)
                    nc.vector.tensor_scalar(t["wy0"][:], t["ly"][:], -1.0, 1.0, OP.mult, OP.add)
                    nc.vector.tensor_scalar(t["mx0"][:], t["ix"][:], -1.0, 1.0, OP.mult, OP.add)
                    nc.vector.tensor_scalar(t["my0"][:], t["iy"][:], -1.0, 1.0, OP.mult, OP.add)
                    nc.vector.tensor_tensor(t["ta"][:], t["msm"][:], t["wy0"][:], OP.mult)
                    nc.vector.tensor_tensor(t["tb"][:], t["msm"][:], t["ly"][:], OP.mult)
                    # separable window factors: window (u0, v0) gets Y_{u0}*X_{v0}
                    #   Y0 = ta*iy, Y1 = ta*my0 + tb*iy, Y2 = tb*my0
                    #   X0 = wx0*ix, X1 = wx0*mx0 + lx*ix, X2 = lx*mx0
                    nc.vector.tensor_tensor(t["Y0"][:], t["ta"][:], t["iy"][:], OP.mult)
                    nc.vector.tensor_tensor(t["Y2"][:], t["tb"][:], t["my0"][:], OP.mult)
                    nc.vector.tensor_tensor(t["Y1"][:], t["ta"][:], t["my0"][:], OP.mult)
                    nc.vector.tensor_tensor(t["tmp"][:], t["tb"][:], t["iy"][:], OP.mult)
                    nc.vector.tensor_tensor(t["Y1"][:], t["Y1"][:], t["tmp"][:], OP.add)
                    nc.vector.tensor_tensor(t["X0"][:], t["wx0"][:], t["ix"][:], OP.mult)
                    nc.vector.tensor_tensor(t["X2"][:], t["lx"][:], t["mx0"][:], OP.mult)
                    nc.vector.tensor_tensor(t["X1"][:], t["wx0"][:], t["mx0"][:], OP.mult)
                    nc.vector.tensor_tensor(t["tmp"][:], t["lx"][:], t["ix"][:], OP.mult)
                    nc.vector.tensor_tensor(t["X1"][:], t["X1"][:], t["tmp"][:], OP.add)

                def emit_cf_group(g0, GRP):
                    # bilinear 5x5 accumulation for tiles [g0, g0+GRP):
                    # 9 separable windows, each Y_{u0}*X_{v0} scattered at
                    # cells (u0 + j, v0 + i)
                    t = cf_t

                    def sl(nm):
                        return t[nm][:, g0:g0 + GRP, :]

                    k = 0
                    for u0 in range(3):
                        for v0 in range(3):
                            cb = t[f"contrib{k % 2}"]
                            k += 1
                            nc.vector.tensor_tensor(cb[:, g0:g0 + GRP, :],
                                                    sl(f"Y{u0}"), sl(f"X{v0}"), OP.mult)
                            # C5[:, :, u0 + j, v0 + i] += Y*X [i, j]
                            dstv = AP(cbuf.tensor,
                                      cbuf[:].offset + 26 * g0 + (u0 * 5 + v0),
                                      [[NT * 26, 128], [26, GRP], [1, 3], [5, 3]])
                            srcv = AP(cb.tensor, cb[:].offset + 9 * g0,
                                      [[NT * 9, 128], [9, GRP], [3, 3], [1, 3]])
                            nc.vector.tensor_tensor(dstv, dstv, srcv, OP.add)

                def emit_f_tile(T, psT, psZ):
                    mt = mtp.tile([128, MTW], bf16, tag="mt")
                    nc.gpsimd.local_scatter(mt[:], cbuf[:, T, :],
                                            sidx[:], channels=128, num_elems=MTW,
                                            num_idxs=26)
                    msb = msbp.tile([128, NCI, 128], bf16, tag="msb")
                    pst = psT.tile([128, 512], bf16, tag="pst")
                    for ci in range(NCI):
                        nc.tensor.transpose(pst[:, 128 * ci:128 * ci + 128],
                                            mt[:, 128 * ci:128 * ci + 128], identb[:])
                    if T % 2 == 0:
                        nc.vector.tensor_copy(msb[:], pst[:])
                    else:
                        nc.scalar.copy(msb[:], pst[:])
                    psz = psZ.tile([128, 256], f32, tag="psz")
                    for ci in range(NCI):
                        nc.tensor.matmul(psz[:], msb[:, ci, :], xpm[:, T + ci, :],
                                         start=(ci == 0), stop=False)
                    nc.tensor.matmul(psz[:], ones_row[:, 0:128], t3r[:],
                                     start=False, stop=True)
                    xrt = fin.tile([128, 256], bf16, tag="xrt")
                    nc.sync.dma_start(xrt[:], d_xr.ap()[T])
                    zact = fin.tile([128, 256], f32, tag="zact")
                    nc.scalar.activation(zact[:], psz[:], AF.Silu)
                    osb = fin.tile([128, 256], bf16, tag="osb")
                    nc.vector.tensor_tensor(osb[:], zact[:], xrt[:], OP.add)
                    nc.sync.dma_start(d_out.ap()[T], osb[:])

                # D3 pairs interleaved with C chunks and E tiles (C's matmuls
                # keep the PE busy while the DVE normalizes)
                ptr_c = 0
                for T0 in range(0, NT, 2):
                    npair = min(2, NT - T0)
                    emit_d3_pair(T0, npair)
                    for _ in range(2):
                        if ptr_c < 24:
                            emit_c_chunk(ptr_c)
                            ptr_c += 1
                    for T in range(T0, T0 + npair):
                        emit_e_tile(T)

                emit_cf_phase1()
                # remaining Xpm chunks: PE filler under phase1's DVE work
                while ptr_c < XCH:
                    emit_c_chunk(ptr_c)
                    ptr_c += 1
                de_ctx.close()
                with tc.tile_pool(name="psT", bufs=2, space="PSUM") as psT, \
                     tc.tile_pool(name="psZ", bufs=3, space="PSUM") as psZ:
                    for ga, gb in GROUPS:
                        emit_cf_group(ga, gb - ga)
                        for T in range(ga, gb):
                            emit_f_tile(T, psT, psZ)

            if dump:
                nc.sync.dma_start(d_dy1.ap(), y1[:])
                nc.sync.dma_start(d_dy.ap(), y[:].rearrange("p a b -> p (a b)"))
                nc.sync.dma_start(d_ddwg.ap(), dw[:].rearrange("p a b -> p (a b)"))
                nc.sync.dma_start(d_dxpm.ap(), xpm[:].rearrange("p a b -> p (a b)"))
                nc.sync.dma_start(d_dcoef.ap(), coefb[:].rearrange("p a b -> p (a b)"))
                nc.sync.dma_start(d_dcbuf.ap(), cbuf[:].rearrange("p a b -> p (a b)"))

    nc.compile()
    return nc


def _get_built():
    global _BUILT
    if _BUILT is None:
        _BUILT = _build()
    return _BUILT


def _bf(a):
    return np.asarray(a, dtype=ml_dtypes.bfloat16)


def _prep(inputs):
    g = {k: np.asarray(v, dtype=np.float32) for k, v in inputs.items()}
    x = g["x"]

    s1 = g["g1"] / np.sqrt(g["v1"] + EPS_BN)
    t1 = g["b1"] - g["m1"] * s1
    s2 = g["g2"] / np.sqrt(g["v2"] + EPS_BN)
    t2 = g["b2"] - g["m2"] * s2
    s3 = g["g3"] / np.sqrt(g["v3"] + EPS_BN)
    t3 = g["b3"] - g["m3"] * s3

    w1 = g["w1"]  # [128, 256, 3, 3]
    w1t = np.zeros((9, 2, 128, 128), np.float32)
    for ki in range(3):
        for kj in range(3):
            for ck in range(2):
                w1t[ki * 3 + kj, ck] = w1[:, 128 * ck:128 * ck + 128, ki, kj].T
    w2t = g["w2"][:, :, 0, 0].T.copy()  # [128, 256]
    Wc = g["out_w"] @ g["in_w"]
    wct = np.stack([Wc.T[:128], Wc.T[128:]])  # [2, 128, 256]
    bc = (g["out_w"] @ g["in_b"] + g["out_b"])[None, :]  # [1, 256]
    dwdg = np.zeros((2, 9, 128, 128), np.float32)
    for ck in range(2):
        for s in range(9):
            np.fill_diagonal(dwdg[ck, s], g["dw_w"][128 * ck:128 * ck + 128, 0, s // 3, s % 3])
    dwb = np.zeros((2, 1, 128), np.float32)
    dwb[0, 0] = g["dw_b"][:128]
    dwb[1, 0] = g["dw_b"][128:]
    womt = np.zeros((2, 128, 32), np.float32)
    for ck in range(2):
        womt[ck, :, :18] = g["off_w"][:, 128 * ck:128 * ck + 128].T
        womt[ck, :, 18:27] = g["msk_w"][:, 128 * ck:128 * ck + 128].T
    ombb = np.zeros((128, 32), np.float32)
    ombb[:, :18] = g["off_b"][None, :]
    ombb[:, 18:27] = g["msk_b"][None, :]
    ident = np.eye(128, dtype=np.float32)

    def colsplit(v):  # [256] -> [2, 128, 1]
        return v.reshape(2, 128, 1).astype(np.float32)

    # fold BN3 scale into the combined projection; t3 added on-chip
    wct = (wct.reshape(2, 128, 256) * s3[None, None, :]).astype(np.float32)
    bc = (bc * s3[None, :]).astype(np.float32)

    selm = np.zeros((32, NT * 128), np.float32)
    for T in range(NT):
        selm[T, 128 * T:128 * T + 128] = 1.0

    shared = dict(
        zeros=np.zeros((128, 512), np.float32),
        onesc=_bf(np.ones((128, 1))),
        onesr=_bf(np.ones((1, 512))),
        selm=_bf(selm),
        w1t=_bf(w1t.transpose(2, 0, 1, 3).copy()), w2t=_bf(w2t),
        wct=_bf(wct.transpose(1, 0, 2).copy()), bcr=_bf(bc),
        dwdg=_bf(dwdg.transpose(2, 0, 1, 3).copy()), dwb=_bf(dwb),
        womt=_bf(womt), ombb=ombb,
        s1=s1[:, None], t1=t1[:, None],
        s2=colsplit(s2), t2=colsplit(t2),
        lng=colsplit(g["ln_g"]), lnb=colsplit(g["ln_b"]),
        ident=ident, identb=_bf(ident),
        t3r=_bf(t3[None, :]),
    )

    in_maps = []
    for c in range(8):
        n, h = c // 2, c % 2
        r0 = 40 * h - 3  # x frame row 0 in global coords
        xs = np.zeros((2, 128, HX, W), np.float32)
        glo = max(r0, 0)
        ghi = min(r0 + HX, 80)
        xs[0, :, glo - r0:ghi - r0, 2:82] = x[n, :128, glo:ghi, :]
        xs[1, :, glo - r0:ghi - r0, 2:82] = x[n, 128:, glo:ghi, :]
        # validity mask for Xpm pixels: q = 128*Q + p -> y-pix q - YL
        vm = np.zeros((XCH * 128,), np.float32)
        qs = np.arange(XCH * 128)
        pix = qs - YL
        rv, cv = pix // W, pix % W
        gr = 40 * h + rv - 2
        ok = (pix >= 0) & (pix < NPY) & (cv >= 2) & (cv < 82) & (gr >= 0) & (gr < 80)
        vm[ok] = 1.0
        vmask = vm.reshape(XCH, 128).T.copy()  # [128, XCH]
        rowm = np.zeros((2, 128, 1), np.float32)
        rowm[0] = 0.0 if h == 0 else 1.0   # y rows [0,2) valid only for h=1
        rowm[1] = 1.0 if h == 0 else 0.0   # y rows [42,44) valid only for h=0
        # pixel-major residual input: xr[T, p, c] = x at out-frame pixel 128T+p
        xflat = np.concatenate([xs[0], xs[1]], 0).reshape(256, HX, W)
        xres = xflat[:, 3:43, :].reshape(256, NPD).T  # [NPD, 256]
        xr = np.zeros((NT * 128, 256), np.float32)
        xr[:NPD] = xres
        m = dict(shared)
        m["xr"] = _bf(xr.reshape(NT, 128, 256))
        m["xs"] = _bf(xs.reshape(2, 128, 4, NPX // 4).transpose(2, 1, 0, 3).copy())
        m["vmask"] = vmask
        m["rowm"] = rowm
        in_maps.append(m)
    return in_maps


def kernel(**inputs):
    nc = _get_built()
    in_maps = _prep(inputs)
    res = bass_utils.run_bass_kernel_spmd(nc, in_maps, core_ids=list(range(8)))
    out = np.zeros((4, 256, 80, 80), np.float32)
    for c in range(8):
        n, h = c // 2, c % 2
        o = np.asarray(res.results[c]["out"], np.float32).reshape(NT * 128, 256)[:NPD]
        o = o.reshape(HD, W, 256)[:, 2:82].transpose(2, 0, 1)
        out[n, :, 40 * h:40 * h + 40, :] = o
    return out
